# revision 1
# baseline (speedup 1.0000x reference)
"""DeepseekV2 decoder layer on 8 TRN2 NeuronCores (Bass/Tile).

Sharding: TP over heads (2/core) for q/kv_b/attention/o_proj, kv_a replicated,
TP over INTER (1024/core) for the MLP. Chunked AllReduce after o_proj and
chunked ReduceScatter after down_proj, overlapped with compute.

Internal layout is feature-major ("transposed"): activations live as
[feature, token] so every matmul output feeds the next as `rhs` without any
on-device transpose. RoPE pair-swaps, RMSNorm weight folding, the softmax
scaling, and cos/sin tables are all folded into host-side weight prep.
"""

import numpy as np
import ml_dtypes

import concourse.bass as bass
import concourse.mybir as mybir
import concourse.tile as tile
from concourse import bacc
from concourse.bass_utils import run_bass_kernel_spmd

BF = ml_dtypes.bfloat16

B, S, HID = 2, 1024, 2048
T = B * S                      # 2048 tokens
H = 16
DN, DR = 128, 64
DQK = DN + DR
DV = 128
KVR = 512
INTER = 8192
EPS = 1e-6
ROPE_BASE = 10000.0
SCALING = DQK ** -0.5

NC_N = 8
HPC = H // NC_N                # 2 heads per core
FPC = INTER // NC_N            # 1024 inter per core
P = 128
HCH = HID // P                 # 16 hid chunks
TT = 4                         # token chunks of 512
TW = T // TT                   # 512
KT = S // P                    # 8 k-tiles of 128 per batch
QT = S // TW                   # 2 q-chunks of 512 per batch
NEG = -30000.0

f32 = mybir.dt.float32
bf16 = mybir.dt.bfloat16
ADD = mybir.AluOpType.add
MUL = mybir.AluOpType.mult
AF = mybir.ActivationFunctionType

_CACHE = {}


def _build():
    nc = bacc.Bacc("TRN2", target_bir_lowering=False, debug=False, num_devices=NC_N)
    dp = lambda n, sh, dt: nc.dram_tensor(n, sh, dt, kind="ExternalInput")
    ht32 = dp("ht32", [HID, T], f32)
    htb = dp("htb", [HID, T], bf16)
    wq = dp("wq", [HID, HPC * DQK], bf16)       # [h0n,h1n,h0x1,h0x2,h1x1,h1x2]
    wkva = dp("wkva", [HID, KVR + DR], bf16)
    wkvbn = dp("wkvbn", [KVR, HPC * DN], bf16)
    wkvbv = dp("wkvbv", [KVR, HPC * DV], bf16)
    wo = dp("wo", [HPC * DV, HID], bf16)
    wg = dp("wg", [HID, FPC], bf16)
    wu = dp("wu", [HID, FPC], bf16)
    wd = dp("wd", [FPC, HID], bf16)
    cosf = dp("cosf", [P, T], bf16)
    sinf = dp("sinf", [P, T], bf16)
    masks = dp("masks", [P, 4, TW], f32)
    out = nc.dram_tensor("o", [HID // NC_N, T], f32, kind="ExternalOutput")
    rg = [list(range(NC_N))]

    with tile.TileContext(nc) as tc:
        with tc.tile_pool(name="const", bufs=1) as cpool, \
             tc.tile_pool(name="dram", bufs=1, space="DRAM") as dram:
            ones_col = cpool.tile([P, 1], bf16)
            nc.vector.memset(ones_col[:], 1.0)
            ones_row = cpool.tile([1, P], bf16)
            nc.vector.memset(ones_row[:], 1.0)
            epsb = cpool.tile([1, 1], f32)
            nc.vector.memset(epsb[:], EPS)

            ar_in = [dram.tile([HID, TW], bf16, name=f"ar_in{t}") for t in range(TT)]
            ar_out = [dram.tile([HID, TW], bf16, addr_space="Shared", name=f"ar_out{t}")
                      for t in range(TT)]
            rs_in = [dram.tile([HID, TW], bf16, name=f"rs_in{t}") for t in range(TT)]
            rs_out = [dram.tile([HID // NC_N, TW], bf16, name=f"rs_out{t}")
                      for t in range(TT)]

            # ============ Phase A: projections + attention ============
            with tc.tile_pool(name="akeep", bufs=1) as akeep, \
                 tc.tile_pool(name="awrk", bufs=3) as awrk, \
                 tc.tile_pool(name="arow", bufs=2) as arow, \
                 tc.tile_pool(name="aps", bufs=1, space="PSUM") as aps:

                # survives both sub-phases
                qsb = akeep.tile([P, 3, T], bf16)          # 12K
                lat = akeep.tile([P, KVR // P, T], bf16)   # 16K
                kpe = akeep.tile([DR, T], bf16)            # 4K
                kva = akeep.tile([P, KVR // P, T], bf16)   # 16K
                bc1 = akeep.tile([P, TT, TW], f32)         # 8K
                bc2 = akeep.tile([P, TT, TW], bf16)        # 4K

                # ---- A1: input norm + q/kv_a projections (ht resident) ----
                with tc.tile_pool(name="atmp1", bufs=1) as a1:
                    ht = a1.tile([P, HCH, T], bf16)        # 64K
                    for o in range(HCH):
                        nc.sync.dma_start(ht[:, o, :], htb.ap()[o * P:(o + 1) * P, :])
                    wq_sb = a1.tile([P, HCH, HPC * DQK], bf16)   # 12K
                    for o in range(HCH):
                        nc.sync.dma_start(wq_sb[:, o, :], wq.ap()[o * P:(o + 1) * P, :])
                    wkva_sb = a1.tile([P, HCH, KVR + DR], bf16)  # 18K
                    for o in range(HCH):
                        nc.sync.dma_start(wkva_sb[:, o, :],
                                          wkva.ap()[o * P:(o + 1) * P, :])

                    # input rmsnorm scale r1[t], broadcast to 128 partitions
                    for t in range(TT):
                        ssp = aps.tile([1, TW], f32, tag="ss", bufs=1, name="ssp")
                        for o in range(HCH):
                            sq = awrk.tile([P, TW], bf16, tag="sq", name="sq")
                            nc.scalar.square(sq[:], ht[:, o, t * TW:(t + 1) * TW])
                            nc.tensor.matmul(ssp[:], ones_col[:], sq[:],
                                             start=(o == 0), stop=(o == HCH - 1))
                        srow = arow.tile([1, TW], f32, tag="srow", name="srow")
                        nc.scalar.activation(srow[:], ssp[:], AF.Sqrt,
                                             bias=epsb[:], scale=1.0 / HID)
                        rrow = arow.tile([1, TW], f32, tag="rrow", name="rrow")
                        nc.vector.reciprocal(rrow[:], srow[:])
                        rb = arow.tile([1, TW], bf16, tag="rb", name="rb")
                        nc.vector.tensor_copy(out=rb[:], in_=rrow[:])
                        bcp = aps.tile([P, TW], f32, tag="big", bufs=2, name="bcp")
                        nc.tensor.matmul(bcp[:], ones_row[:], rb[:],
                                         start=True, stop=True)
                        nc.vector.tensor_copy(out=bc1[:, t, :], in_=bcp[:])

                    # q projection (scaled by r1; SCALING folded into wq on host)
                    for f in range(3):
                        for t in range(TT):
                            qp = aps.tile([P, TW], f32, tag="big", bufs=2, name="qp")
                            for o in range(HCH):
                                nc.tensor.matmul(qp[:], wq_sb[:, o, f * P:(f + 1) * P],
                                                 ht[:, o, t * TW:(t + 1) * TW],
                                                 start=(o == 0), stop=(o == HCH - 1))
                            nc.vector.tensor_tensor(qsb[:, f, t * TW:(t + 1) * TW],
                                                    qp[:], bc1[:, t, :], MUL)

                    # kv_a: latent raw (2nd norm is scale-invariant) + k_pe*r1
                    for t in range(TT):
                        ss2p = aps.tile([1, TW], f32, tag="ss2", bufs=1, name="ss2p")
                        for f in range(KVR // P + 1):
                            wid = P if f < KVR // P else DR
                            lp = aps.tile([P, TW], f32, tag="big", bufs=2, name="lp")
                            for o in range(HCH):
                                nc.tensor.matmul(lp[:wid, :],
                                                 wkva_sb[:, o, f * P:f * P + wid],
                                                 ht[:, o, t * TW:(t + 1) * TW],
                                                 start=(o == 0), stop=(o == HCH - 1))
                            if f < KVR // P:
                                nc.vector.tensor_copy(
                                    out=lat[:, f, t * TW:(t + 1) * TW], in_=lp[:])
                                sq2 = awrk.tile([P, TW], bf16, tag="sq", name="sq2")
                                nc.scalar.square(sq2[:], lp[:])
                                nc.tensor.matmul(ss2p[:], ones_col[:], sq2[:],
                                                 start=(f == 0),
                                                 stop=(f == KVR // P - 1))
                            else:
                                nc.vector.tensor_tensor(kpe[:, t * TW:(t + 1) * TW],
                                                        lp[:DR, :], bc1[:DR, t, :],
                                                        MUL)
                        srow2 = arow.tile([1, TW], f32, tag="srow", name="srow2")
                        nc.scalar.activation(srow2[:], ss2p[:], AF.Sqrt,
                                             bias=epsb[:], scale=1.0 / KVR)
                        rrow2 = arow.tile([1, TW], f32, tag="rrow", name="rrow2")
                        nc.vector.reciprocal(rrow2[:], srow2[:])
                        rb2 = arow.tile([1, TW], bf16, tag="rb", name="rb2")
                        nc.vector.tensor_copy(out=rb2[:], in_=rrow2[:])
                        bcp2 = aps.tile([P, TW], f32, tag="big", bufs=2, name="bcp2")
                        nc.tensor.matmul(bcp2[:], ones_row[:], rb2[:],
                                         start=True, stop=True)
                        nc.vector.tensor_copy(out=bc2[:, t, :], in_=bcp2[:])
                        for f in range(KVR // P):
                            nc.vector.tensor_tensor(kva[:, f, t * TW:(t + 1) * TW],
                                                    lat[:, f, t * TW:(t + 1) * TW],
                                                    bc2[:, t, :], MUL)

                # ---- A2: rope, kv_b, attention, o_proj (+AR) ----
                with tc.tile_pool(name="atmp2", bufs=1) as a2:
                    cs = a2.tile([P, T], bf16)
                    nc.sync.dma_start(cs[:], cosf.ap())
                    sn = a2.tile([P, T], bf16)
                    nc.sync.dma_start(sn[:], sinf.ap())
                    msk = a2.tile([P, 4, TW], f32)
                    nc.sync.dma_start(msk[:], masks.ap())
                    wkvbn_sb = a2.tile([P, KVR // P, HPC * DN], bf16)
                    for o in range(KVR // P):
                        nc.sync.dma_start(wkvbn_sb[:, o, :],
                                          wkvbn.ap()[o * P:(o + 1) * P, :])
                    wkvbv_sb = a2.tile([P, KVR // P, HPC * DV], bf16)
                    for o in range(KVR // P):
                        nc.sync.dma_start(wkvbv_sb[:, o, :],
                                          wkvbv.ap()[o * P:(o + 1) * P, :])
                    wo_sb = a2.tile([P, HPC, HID], bf16)
                    for h in range(HPC):
                        nc.sync.dma_start(wo_sb[:, h, :], wo.ap()[h * P:(h + 1) * P, :])

                    # rope: [x1(32); x2(32)] per head; pair-swap via sbuf dma
                    qrope = []
                    for h in range(HPC):
                        src = qsb[:, 2, :]
                        if h == 0:
                            direct = src[0:DR, :]
                        else:
                            dcp = awrk.tile([DR, T], bf16, tag="rope", name="dcp")
                            nc.sync.dma_start(dcp[:], src[DR:2 * DR, :])
                            direct = dcp[:]
                        sw = awrk.tile([DR, T], bf16, tag="rope", name="qsw")
                        nc.sync.dma_start(sw[0:32, :], src[h * DR + 32:h * DR + 64, :])
                        nc.sync.dma_start(sw[32:64, :], src[h * DR:h * DR + 32, :])
                        qr = a2.tile([DR, T], bf16, name=f"qr{h}")
                        tmp = awrk.tile([DR, T], bf16, tag="rope", name="qtmp")
                        nc.vector.tensor_tensor(tmp[:], direct, cs[0:DR, :], MUL)
                        nc.vector.tensor_tensor(qr[:], sw[:], sn[0:DR, :], MUL)
                        nc.vector.tensor_tensor(qr[:], qr[:], tmp[:], ADD)
                        qrope.append(qr)
                    ksw = awrk.tile([DR, T], bf16, tag="rope", name="ksw")
                    nc.sync.dma_start(ksw[0:32, :], kpe[32:64, :])
                    nc.sync.dma_start(ksw[32:64, :], kpe[0:32, :])
                    krope = a2.tile([DR, T], bf16)
                    ktmp = awrk.tile([DR, T], bf16, tag="rope", name="ktmp")
                    nc.vector.tensor_tensor(ktmp[:], kpe[:], cs[0:DR, :], MUL)
                    nc.vector.tensor_tensor(krope[:], ksw[:], sn[0:DR, :], MUL)
                    nc.vector.tensor_tensor(krope[:], krope[:], ktmp[:], ADD)

                    # kv_b: k_nope (transposed out) + v (natural out)
                    knope = a2.tile([P, HPC, T], bf16)
                    for h in range(HPC):
                        for t in range(TT):
                            kp = aps.tile([P, TW], f32, tag="big", bufs=2, name="kp")
                            for c in range(KVR // P):
                                nc.tensor.matmul(kp[:],
                                                 wkvbn_sb[:, c, h * P:(h + 1) * P],
                                                 kva[:, c, t * TW:(t + 1) * TW],
                                                 start=(c == 0),
                                                 stop=(c == KVR // P - 1))
                            nc.vector.tensor_copy(out=knope[:, h, t * TW:(t + 1) * TW],
                                                  in_=kp[:])
                    vnat = a2.tile([P, T // P, HPC * DV], bf16)
                    for to in range(T // P):
                        vp = aps.tile([P, HPC * DV], f32, tag="vp", bufs=1, name="vp")
                        for c in range(KVR // P):
                            nc.tensor.matmul(vp[:], kva[:, c, to * P:(to + 1) * P],
                                             wkvbv_sb[:, c, :],
                                             start=(c == 0), stop=(c == KVR // P - 1))
                        nc.vector.tensor_copy(out=vnat[:, to, :], in_=vp[:])

                    # attention (scores transposed: [k, q]) + o_proj partial + AR
                    attn = a2.tile([P, HPC, T], bf16)
                    for b in range(B):
                        for qt in range(QT):
                            tt = b * QT + qt
                            qc0 = b * S + qt * TW
                            nkt = 4 * qt + 4
                            for h in range(HPC):
                                dnp = aps.tile([1, TW], f32, tag="den", bufs=1, name="dnp")
                                atp = aps.tile([P, TW], f32, tag="att", bufs=2, name="atp")
                                exs = [None] * nkt

                                def consume(kt):
                                    nc.tensor.matmul(dnp[:], ones_col[:], exs[kt][:],
                                                     start=(kt == 0),
                                                     stop=(kt == nkt - 1))
                                    nc.tensor.matmul(atp[:],
                                                     vnat[:, b * KT + kt,
                                                          h * DV:(h + 1) * DV],
                                                     exs[kt][:],
                                                     start=(kt == 0),
                                                     stop=(kt == nkt - 1))

                                for kt in range(nkt):
                                    kc0 = b * S + kt * P
                                    scp = aps.tile([P, TW], f32, tag="big", bufs=2, name="scp")
                                    nc.tensor.matmul(scp[:],
                                                     knope[:, h, kc0:kc0 + P],
                                                     qsb[:, h, qc0:qc0 + TW],
                                                     start=True, stop=False)
                                    nc.tensor.matmul(scp[:],
                                                     krope[:, kc0:kc0 + P],
                                                     qrope[h][:, qc0:qc0 + TW],
                                                     start=False, stop=True)
                                    ex = awrk.tile([P, TW], bf16, tag="ex", bufs=4,
                                                   name="ex")
                                    j = kt - 4 * qt
                                    if j >= 0:
                                        mtmp = awrk.tile([P, TW], f32, tag="mt",
                                                         name="mtmp")
                                        nc.vector.tensor_tensor(mtmp[:], scp[:],
                                                                msk[:, j, :], ADD)
                                        nc.scalar.activation(ex[:], mtmp[:], AF.Exp)
                                    else:
                                        nc.scalar.activation(ex[:], scp[:], AF.Exp)
                                    exs[kt] = ex
                                    if kt >= 2:
                                        consume(kt - 2)
                                consume(max(nkt - 2, 0))
                                if nkt > 1:
                                    consume(nkt - 1)
                                drow = arow.tile([1, TW], bf16, tag="rb", name="drow")
                                with nc.allow_low_precision(reason="softmax denom"):
                                    nc.vector.reciprocal(drow[:], dnp[:])
                                dbp = aps.tile([P, TW], f32, tag="big", bufs=2, name="dbp")
                                nc.tensor.matmul(dbp[:], ones_row[:], drow[:],
                                                 start=True, stop=True)
                                dbc = awrk.tile([P, TW], f32, tag="mt", name="dbc")
                                nc.vector.tensor_copy(out=dbc[:], in_=dbp[:])
                                nc.vector.tensor_tensor(
                                    attn[:, h, qc0:qc0 + TW], atp[:], dbc[:], MUL)
                            # o_proj partial for this token chunk
                            for ho in range(HCH):
                                op = aps.tile([P, TW], f32, tag="big", bufs=2, name="op")
                                for h in range(HPC):
                                    nc.tensor.matmul(op[:],
                                                     wo_sb[:, h, ho * P:(ho + 1) * P],
                                                     attn[:, h, qc0:qc0 + TW],
                                                     start=(h == 0),
                                                     stop=(h == HPC - 1))
                                osb = awrk.tile([P, TW], bf16, tag="ex", bufs=4, name="osb")
                                nc.vector.tensor_copy(out=osb[:], in_=op[:])
                                nc.sync.dma_start(ar_in[tt][ho * P:(ho + 1) * P, :],
                                                  osb[:])
                            nc.gpsimd.collective_compute(
                                "AllReduce", ADD, ins=[ar_in[tt][:].opt()],
                                outs=[ar_out[tt][:].opt()], replica_groups=rg)

            # ============ Phase B: residual + norm + MLP ============
            with tc.tile_pool(name="bbig", bufs=1) as bbig, \
                 tc.tile_pool(name="bwrk", bufs=3) as bwrk, \
                 tc.tile_pool(name="brow", bufs=1) as brow, \
                 tc.tile_pool(name="bps", bufs=1, space="PSUM") as bps:

                wg_sb = bbig.tile([P, HCH, FPC], bf16)       # 32K
                for o in range(HCH):
                    nc.sync.dma_start(wg_sb[:, o, :], wg.ap()[o * P:(o + 1) * P, :])
                wu_sb = bbig.tile([P, HCH, FPC], bf16)       # 32K
                for o in range(HCH):
                    nc.sync.dma_start(wu_sb[:, o, :], wu.ap()[o * P:(o + 1) * P, :])
                wd_sb = bbig.tile([P, FPC // P, HID], bf16)  # 16K
                for o in range(FPC // P):
                    nc.sync.dma_start(wd_sb[:, o, :], wd.ap()[o * P:(o + 1) * P, :])

                for t in range(TT):
                    # x = hidden + attn_out; later scaled in place to x/8
                    x = bbig.tile([P, HCH, TW], bf16, name="x", tag="x", bufs=2)
                    ssp3 = bps.tile([1, TW], f32, tag="ss", bufs=1, name="ssp3")
                    for o in range(HCH):
                        harp = bwrk.tile([P, TW], f32, tag="harp", name="harp")
                        nc.sync.dma_start(
                            harp[:], ht32.ap()[o * P:(o + 1) * P, t * TW:(t + 1) * TW])
                        arsb = bwrk.tile([P, TW], bf16, tag="arsb", bufs=3, name="arsb")
                        nc.sync.dma_start(arsb[:], ar_out[t][o * P:(o + 1) * P, :])
                        nc.vector.tensor_tensor(x[:, o, :], harp[:], arsb[:], ADD)
                        sq3 = bwrk.tile([P, TW], bf16, tag="sq3", bufs=2, name="sq3")
                        nc.scalar.square(sq3[:], x[:, o, :])
                        nc.tensor.matmul(ssp3[:], ones_col[:], sq3[:],
                                         start=(o == 0), stop=(o == HCH - 1))
                    srow3 = brow.tile([1, TW], f32, tag="srow3", name="srow3")
                    nc.scalar.activation(srow3[:], ssp3[:], AF.Sqrt,
                                         bias=epsb[:], scale=1.0 / HID)
                    rrow3 = brow.tile([1, TW], f32, tag="rrow3", name="rrow3")
                    nc.vector.reciprocal(rrow3[:], srow3[:])
                    rb3 = brow.tile([1, TW], bf16, tag="rb3", name="rb3")
                    nc.vector.tensor_copy(out=rb3[:], in_=rrow3[:])
                    bcp3 = bps.tile([P, TW], f32, tag="gu", bufs=4, name="bcp3")
                    nc.tensor.matmul(bcp3[:], ones_row[:], rb3[:], start=True, stop=True)
                    bc3 = bwrk.tile([P, TW], f32, tag="bc3", bufs=1, name="bc3")
                    nc.vector.tensor_copy(out=bc3[:], in_=bcp3[:])
                    h2 = bbig.tile([P, HCH, TW], bf16, name="h2", tag="h2", bufs=2)
                    for o in range(HCH):
                        nc.vector.tensor_tensor(h2[:, o, :], x[:, o, :], bc3[:], MUL)
                        # x -> x/8 in place (folded residual for ReduceScatter)
                        nc.vector.tensor_scalar_mul(x[:, o, :], x[:, o, :], 0.125)

                    # gate/up/silu
                    act = bbig.tile([P, FPC // P, TW], bf16, name="act", tag="act",
                                    bufs=1)
                    for fi in range(FPC // P):
                        gp = bps.tile([P, TW], f32, tag="gu", bufs=4, name="gp")
                        for o in range(HCH):
                            nc.tensor.matmul(gp[:], wg_sb[:, o, fi * P:(fi + 1) * P],
                                             h2[:, o, :],
                                             start=(o == 0), stop=(o == HCH - 1))
                        up = bps.tile([P, TW], f32, tag="gu", bufs=4, name="up")
                        for o in range(HCH):
                            nc.tensor.matmul(up[:], wu_sb[:, o, fi * P:(fi + 1) * P],
                                             h2[:, o, :],
                                             start=(o == 0), stop=(o == HCH - 1))
                        gs = bwrk.tile([P, TW], f32, tag="gs", bufs=2, name="gs")
                        nc.scalar.activation(gs[:], gp[:], AF.Silu)
                        nc.vector.tensor_tensor(act[:, fi, :], up[:], gs[:], MUL)

                    # down projection partial (+x/8) + RS
                    for ho in range(HCH):
                        dpp = bps.tile([P, TW], f32, tag="d", bufs=2, name="dpp")
                        for c in range(FPC // P):
                            nc.tensor.matmul(dpp[:], wd_sb[:, c, ho * P:(ho + 1) * P],
                                             act[:, c, :],
                                             start=(c == 0), stop=(c == FPC // P - 1))
                        dsb = bwrk.tile([P, TW], bf16, tag="dsb", bufs=3, name="dsb")
                        nc.vector.tensor_tensor(dsb[:], dpp[:], x[:, ho, :], ADD)
                        nc.sync.dma_start(rs_in[t][ho * P:(ho + 1) * P, :], dsb[:])
                    nc.gpsimd.collective_compute(
                        "ReduceScatter", ADD, ins=[rs_in[t][:].opt()],
                        outs=[rs_out[t][:].opt()], replica_groups=rg)
                    for o in range(HID // NC_N // P):
                        fin = bwrk.tile([P, TW], bf16, tag="fin", bufs=2, name="fin")
                        nc.sync.dma_start(fin[:], rs_out[t][o * P:(o + 1) * P, :])
                        finf = bwrk.tile([P, TW], f32, tag="finf", bufs=2, name="finf")
                        nc.vector.tensor_copy(out=finf[:], in_=fin[:])
                        nc.sync.dma_start(
                            out.ap()[o * P:(o + 1) * P, t * TW:(t + 1) * TW], finf[:])
    nc.compile()
    return nc


def _prep(hidden_states, positions, w_in_ln, w_q, w_kv_a, w_kv_a_ln,
          w_kv_b, w_o, w_post_ln, w_gate, w_up, w_down):
    hT = np.ascontiguousarray(
        np.asarray(hidden_states, np.float32).reshape(T, HID).T)

    pos = np.asarray(positions).reshape(-1).astype(np.float64)
    inv = ROPE_BASE ** (-np.arange(0, DR, 2, dtype=np.float64) / DR)
    fr = pos[:, None] * inv[None, :]                      # [T, 32]
    c32 = np.cos(fr).T.astype(np.float32)                 # [32, T]
    s32 = np.sin(fr).T.astype(np.float32)
    cosf = np.concatenate([c32] * 4, 0)
    sinf = np.concatenate([-s32, s32, -s32, s32], 0)

    r = np.arange(P)[:, None]
    c = np.arange(TW)[None, :]
    masks = np.stack([np.where(c >= r + j * P, 0.0, NEG) for j in range(4)],
                     1).astype(np.float32)                # [128, 4, 512]

    w_in_ln = np.asarray(w_in_ln, np.float32)
    wqf = (np.asarray(w_q, np.float32) * w_in_ln[:, None] * SCALING
           ).reshape(HID, H, DQK)
    wkvaf = np.asarray(w_kv_a, np.float32) * w_in_ln[:, None]
    kpe_w = wkvaf[:, KVR:]
    wkva_p = np.concatenate([wkvaf[:, :KVR], kpe_w[:, 0::2], kpe_w[:, 1::2]], 1)
    wkvbf = (np.asarray(w_kv_b, np.float32)
             * np.asarray(w_kv_a_ln, np.float32)[:, None]).reshape(KVR, H, DN + DV)
    w_post_ln = np.asarray(w_post_ln, np.float32)
    wgf = np.asarray(w_gate, np.float32) * w_post_ln[:, None]
    wuf = np.asarray(w_up, np.float32) * w_post_ln[:, None]
    wdf = np.asarray(w_down, np.float32)
    wof = np.asarray(w_o, np.float32).reshape(H, DV, HID)

    in_maps = []
    for core in range(NC_N):
        hs = [2 * core, 2 * core + 1]
        nopes = np.concatenate([wqf[:, h, :DN] for h in hs], 1)
        pes = []
        for h in hs:
            pe = wqf[:, h, DN:]
            pes += [pe[:, 0::2], pe[:, 1::2]]
        wq_c = np.concatenate([nopes] + pes, 1)
        in_maps.append({
            "ht32": hT,
            "htb": hT.astype(BF),
            "wq": wq_c.astype(BF),
            "wkva": wkva_p.astype(BF),
            "wkvbn": np.concatenate([wkvbf[:, h, :DN] for h in hs], 1).astype(BF),
            "wkvbv": np.concatenate([wkvbf[:, h, DN:] for h in hs], 1).astype(BF),
            "wo": np.concatenate([wof[h] for h in hs], 0).astype(BF),
            "wg": wgf[:, core * FPC:(core + 1) * FPC].astype(BF),
            "wu": wuf[:, core * FPC:(core + 1) * FPC].astype(BF),
            "wd": wdf[core * FPC:(core + 1) * FPC, :].astype(BF),
            "cosf": cosf.astype(BF),
            "sinf": sinf.astype(BF),
            "masks": masks,
        })
    return in_maps


def kernel(**inputs):
    if "nc" not in _CACHE:
        _CACHE["nc"] = _build()
    nc = _CACHE["nc"]
    in_maps = _prep(**inputs)
    res = run_bass_kernel_spmd(nc, in_maps, core_ids=list(range(NC_N)))
    outT = np.concatenate([res.results[c]["o"] for c in range(NC_N)], 0)
    return np.ascontiguousarray(outT.T).reshape(B, S, HID).astype(np.float32)



# revision 9
# speedup vs baseline: 1.0227x; 1.0227x over previous
"""DeepseekV2 decoder layer on 8 TRN2 NeuronCores (Bass/Tile).

Sharding: TP over heads (2/core) for q/kv_b/attention/o_proj, kv_a replicated,
TP over INTER (1024/core) for the MLP. Chunked AllReduce after o_proj and
chunked ReduceScatter after down_proj, overlapped with compute.

Internal layout is feature-major ("transposed"): activations live as
[feature, token] so every matmul output feeds the next as `rhs` without any
on-device transpose. RoPE pair-swaps, RMSNorm weight folding, the softmax
scaling, and cos/sin tables are all folded into host-side weight prep.

Perf notes vs v1:
- all 1-partition RECIPROCALs (3.3us each) replaced by broadcast-first then
  [128,512] reciprocal (~0.6us), norm scales stay bf16 end-to-end
- residual comes from the bf16 htb (8MB) instead of a separate f32 copy (16MB)
- MLP weights prefetched during attention (kills the phase A->B dead zone)
- kva written in-place over lat (frees 16KB/partition for the prefetch)
- bf16 operand discipline for DVE 2x mode; PSUM-side copies moved to ScalarE
- x/8 fold fused into the down-residual via scalar_tensor_tensor
"""

import numpy as np
import ml_dtypes

import concourse.bass as bass
import concourse.mybir as mybir
import concourse.tile as tile
from concourse import bacc
from concourse.bass_utils import run_bass_kernel_spmd

BF = ml_dtypes.bfloat16

B, S, HID = 2, 1024, 2048
T = B * S                      # 2048 tokens
H = 16
DN, DR = 128, 64
DQK = DN + DR
DV = 128
KVR = 512
INTER = 8192
EPS = 1e-6
ROPE_BASE = 10000.0
SCALING = DQK ** -0.5

NC_N = 8
HPC = H // NC_N                # 2 heads per core
FPC = INTER // NC_N            # 1024 inter per core
P = 128
HCH = HID // P                 # 16 hid chunks
TT = 4                         # token chunks of 512
TW = T // TT                   # 512
KT = S // P                    # 8 k-tiles of 128 per batch
QT = S // TW                   # 2 q-chunks of 512 per batch
NEG = -30000.0

f32 = mybir.dt.float32
bf16 = mybir.dt.bfloat16
ADD = mybir.AluOpType.add
MUL = mybir.AluOpType.mult
AF = mybir.ActivationFunctionType

_CACHE = {}


def _build():
    nc = bacc.Bacc("TRN2", target_bir_lowering=False, debug=False, num_devices=NC_N)
    dp = lambda n, sh, dt: nc.dram_tensor(n, sh, dt, kind="ExternalInput")
    htb = dp("htb", [HID, T], bf16)
    wq = dp("wq", [HID, HPC * DQK], bf16)       # [h0n,h1n,h0x1,h0x2,h1x1,h1x2]
    wkva = dp("wkva", [HID, KVR + DR], bf16)
    wkvbn = dp("wkvbn", [KVR, HPC * DN], bf16)
    wkvbv = dp("wkvbv", [KVR, HPC * DV], bf16)
    wo = dp("wo", [HPC * DV, HID], bf16)
    wg = dp("wg", [HID, FPC], bf16)
    wu = dp("wu", [HID, FPC], bf16)
    wd = dp("wd", [FPC, HID], bf16)
    cosf = dp("cosf", [P, T], bf16)
    sinf = dp("sinf", [P, T], bf16)
    masks = dp("masks", [P, 4, TW], f32)
    out = nc.dram_tensor("o", [HID // NC_N, T], f32, kind="ExternalOutput")
    rg = [list(range(NC_N))]

    with tile.TileContext(nc) as tc:
        with tc.tile_pool(name="const", bufs=1) as cpool, \
             tc.tile_pool(name="dram", bufs=1, space="DRAM") as dram:
            ones_col = cpool.tile([P, 1], bf16)
            nc.vector.memset(ones_col[:], 1.0)
            ones_row = cpool.tile([1, P], bf16)
            nc.vector.memset(ones_row[:], 1.0)
            epsb = cpool.tile([1, 1], f32)
            nc.vector.memset(epsb[:], EPS)

            ar_in = [dram.tile([HID, TW], bf16, name=f"ar_in{t}") for t in range(TT)]
            ar_out = [dram.tile([HID, TW], bf16, addr_space="Shared", name=f"ar_out{t}")
                      for t in range(TT)]
            rs_in = [dram.tile([HID, TW], bf16, name=f"rs_in{t}") for t in range(TT)]
            rs_out = [dram.tile([HID // NC_N, TW], bf16, name=f"rs_out{t}")
                      for t in range(TT)]

            # ============ Phase A: projections + attention ============
            with tc.tile_pool(name="akeep", bufs=1) as akeep, \
                 tc.tile_pool(name="awrk", bufs=3) as awrk, \
                 tc.tile_pool(name="arow", bufs=2) as arow:

                # survives both sub-phases
                qsb = akeep.tile([P, 3, T], bf16)          # 12K
                lat = akeep.tile([P, KVR // P, T], bf16)   # 16K (kva in place)
                kpe = akeep.tile([DR, T], bf16)            # 4K

                # ---- A1: input norm + q/kv_a projections (ht resident) ----
                with tc.tile_pool(name="atmp1", bufs=1) as a1, \
                     tc.tile_pool(name="aps1", bufs=1, space="PSUM") as aps:
                    wq_sb = a1.tile([P, HCH, HPC * DQK], bf16)   # 12K
                    for o in range(HCH):
                        nc.sync.dma_start(wq_sb[:, o, :], wq.ap()[o * P:(o + 1) * P, :])
                    wkva_sb = a1.tile([P, HCH, KVR + DR], bf16)  # 18K
                    for o in range(HCH):
                        nc.sync.dma_start(wkva_sb[:, o, :],
                                          wkva.ap()[o * P:(o + 1) * P, :])
                    ht = a1.tile([P, HCH, T], bf16)        # 64K
                    for t in range(TT):
                        for o in range(HCH):
                            nc.sync.dma_start(ht[:, o, t * TW:(t + 1) * TW],
                                              htb.ap()[o * P:(o + 1) * P,
                                                       t * TW:(t + 1) * TW])
                    bc1 = a1.tile([P, TT, TW], bf16)       # 4K  (1/rms of input)

                    for t in range(TT):
                        tsl = slice(t * TW, (t + 1) * TW)
                        # input rmsnorm: sum of squares over HID via ones-matmul
                        ssp = aps.tile([1, TW], f32, tag="ss", bufs=2, name="ssp")
                        for o in range(HCH):
                            sq = awrk.tile([P, TW], bf16, tag="sq", name="sq")
                            if o % 2 == 0:
                                nc.scalar.square(sq[:], ht[:, o, tsl])
                            else:
                                nc.vector.tensor_tensor(sq[:], ht[:, o, tsl],
                                                        ht[:, o, tsl], MUL)
                            nc.tensor.matmul(ssp[:], ones_col[:], sq[:],
                                             start=(o == 0), stop=(o == HCH - 1))
                        srow = arow.tile([1, TW], bf16, tag="srow", name="srow")
                        nc.scalar.activation(srow[:], ssp[:], AF.Sqrt,
                                             bias=epsb[:], scale=1.0 / HID)
                        bcp = aps.tile([P, TW], f32, tag="big", bufs=3, name="bcp")
                        nc.tensor.matmul(bcp[:], ones_row[:], srow[:],
                                         start=True, stop=True)
                        with nc.allow_low_precision(reason="rms scale bf16"):
                            nc.vector.reciprocal(bc1[:, t, :], bcp[:])

                        # q projection (scaled by r1; SCALING folded into wq)
                        for f in range(3):
                            qp = aps.tile([P, TW], f32, tag="big", bufs=3, name="qp")
                            for o in range(HCH):
                                nc.tensor.matmul(qp[:], wq_sb[:, o, f * P:(f + 1) * P],
                                                 ht[:, o, tsl],
                                                 start=(o == 0), stop=(o == HCH - 1))
                            nc.vector.tensor_tensor(qsb[:, f, tsl], qp[:],
                                                    bc1[:, t, :], MUL)

                        # kv_a: latent raw (2nd norm is scale-invariant) + k_pe*r1
                        ss2p = aps.tile([1, TW], f32, tag="ss", bufs=2, name="ss2p")
                        for f in range(KVR // P + 1):
                            wid = P if f < KVR // P else DR
                            lp = aps.tile([P, TW], f32, tag="big", bufs=3, name="lp")
                            for o in range(HCH):
                                nc.tensor.matmul(lp[:wid, :],
                                                 wkva_sb[:, o, f * P:f * P + wid],
                                                 ht[:, o, tsl],
                                                 start=(o == 0), stop=(o == HCH - 1))
                            if f < KVR // P:
                                nc.vector.tensor_copy(out=lat[:, f, tsl], in_=lp[:])
                                sq2 = awrk.tile([P, TW], bf16, tag="sq", name="sq2")
                                nc.scalar.square(sq2[:], lp[:])
                                nc.tensor.matmul(ss2p[:], ones_col[:], sq2[:],
                                                 start=(f == 0),
                                                 stop=(f == KVR // P - 1))
                            else:
                                nc.vector.tensor_tensor(kpe[:, tsl], lp[:DR, :],
                                                        bc1[:DR, t, :], MUL)
                        srow2 = arow.tile([1, TW], bf16, tag="srow", name="srow2")
                        nc.scalar.activation(srow2[:], ss2p[:], AF.Sqrt,
                                             bias=epsb[:], scale=1.0 / KVR)
                        bcp2 = aps.tile([P, TW], f32, tag="big", bufs=3, name="bcp2")
                        nc.tensor.matmul(bcp2[:], ones_row[:], srow2[:],
                                         start=True, stop=True)
                        bc2t = a1.tile([P, TW], bf16, tag="bc2", bufs=2, name="bc2t")
                        with nc.allow_low_precision(reason="rms scale bf16"):
                            nc.vector.reciprocal(bc2t[:], bcp2[:])
                        for f in range(KVR // P):
                            nc.vector.tensor_tensor(lat[:, f, tsl], lat[:, f, tsl],
                                                    bc2t[:], MUL)

                # MLP weights: prefetch now so the DMA hides under attention
                bbig = tc.alloc_tile_pool(name="bbig", bufs=1, side="right")
                wg_sb = bbig.tile([P, HCH, FPC], bf16)       # 32K
                for o in range(HCH):
                    nc.sync.dma_start(wg_sb[:, o, :], wg.ap()[o * P:(o + 1) * P, :])
                wu_sb = bbig.tile([P, HCH, FPC], bf16)       # 32K
                for o in range(HCH):
                    nc.sync.dma_start(wu_sb[:, o, :], wu.ap()[o * P:(o + 1) * P, :])

                # ---- A2: rope, kv_b, attention, o_proj (+AR) ----
                with tc.tile_pool(name="atmp2", bufs=1) as a2, \
                     tc.tile_pool(name="aps2", bufs=1, space="PSUM") as aps:
                    cs = a2.tile([P, T], bf16)
                    nc.sync.dma_start(cs[:], cosf.ap())
                    sn = a2.tile([P, T], bf16)
                    nc.sync.dma_start(sn[:], sinf.ap())
                    msk = a2.tile([P, 4, TW], f32)
                    nc.sync.dma_start(msk[:], masks.ap())
                    wkvbn_sb = a2.tile([P, KVR // P, HPC * DN], bf16)
                    for o in range(KVR // P):
                        nc.sync.dma_start(wkvbn_sb[:, o, :],
                                          wkvbn.ap()[o * P:(o + 1) * P, :])
                    wkvbv_sb = a2.tile([P, KVR // P, HPC * DV], bf16)
                    for o in range(KVR // P):
                        nc.sync.dma_start(wkvbv_sb[:, o, :],
                                          wkvbv.ap()[o * P:(o + 1) * P, :])
                    wo_sb = a2.tile([P, HPC, HID], bf16)
                    for h in range(HPC):
                        nc.sync.dma_start(wo_sb[:, h, :], wo.ap()[h * P:(h + 1) * P, :])

                    # rope: [x1(32); x2(32)] per head; pair-swap via sbuf dma
                    qrope = []
                    for h in range(HPC):
                        src = qsb[:, 2, :]
                        if h == 0:
                            direct = src[0:DR, :]
                        else:
                            dcp = awrk.tile([DR, T], bf16, tag="rope", name="dcp")
                            nc.sync.dma_start(dcp[:], src[DR:2 * DR, :])
                            direct = dcp[:]
                        sw = awrk.tile([DR, T], bf16, tag="rope", name="qsw")
                        nc.sync.dma_start(sw[0:32, :], src[h * DR + 32:h * DR + 64, :])
                        nc.sync.dma_start(sw[32:64, :], src[h * DR:h * DR + 32, :])
                        qr = a2.tile([DR, T], bf16, name=f"qr{h}")
                        tmp = awrk.tile([DR, T], bf16, tag="rope", name="qtmp")
                        nc.vector.tensor_tensor(tmp[:], direct, cs[0:DR, :], MUL)
                        nc.vector.tensor_tensor(qr[:], sw[:], sn[0:DR, :], MUL)
                        nc.vector.tensor_tensor(qr[:], qr[:], tmp[:], ADD)
                        qrope.append(qr)
                    ksw = awrk.tile([DR, T], bf16, tag="rope", name="ksw")
                    nc.sync.dma_start(ksw[0:32, :], kpe[32:64, :])
                    nc.sync.dma_start(ksw[32:64, :], kpe[0:32, :])
                    krope = a2.tile([DR, T], bf16)
                    ktmp = awrk.tile([DR, T], bf16, tag="rope", name="ktmp")
                    nc.vector.tensor_tensor(ktmp[:], kpe[:], cs[0:DR, :], MUL)
                    nc.vector.tensor_tensor(krope[:], ksw[:], sn[0:DR, :], MUL)
                    nc.vector.tensor_tensor(krope[:], krope[:], ktmp[:], ADD)

                    # kv_b: k_nope (transposed out) + v (natural out)
                    knope = a2.tile([P, HPC, T], bf16)
                    for h in range(HPC):
                        for t in range(TT):
                            kp = aps.tile([P, TW], f32, tag="big", bufs=3, name="kp")
                            for c in range(KVR // P):
                                nc.tensor.matmul(kp[:],
                                                 wkvbn_sb[:, c, h * P:(h + 1) * P],
                                                 lat[:, c, t * TW:(t + 1) * TW],
                                                 start=(c == 0),
                                                 stop=(c == KVR // P - 1))
                            nc.vector.tensor_copy(out=knope[:, h, t * TW:(t + 1) * TW],
                                                  in_=kp[:])
                    vnat = a2.tile([P, T // P, HPC * DV], bf16)
                    for to in range(T // P):
                        vp = aps.tile([P, HPC * DV], f32, tag="vp", bufs=1, name="vp")
                        for c in range(KVR // P):
                            nc.tensor.matmul(vp[:], lat[:, c, to * P:(to + 1) * P],
                                             wkvbv_sb[:, c, :],
                                             start=(c == 0), stop=(c == KVR // P - 1))
                        nc.scalar.copy(vnat[:, to, :], vp[:])

                    # attention (scores transposed: [k, q]) + o_proj partial + AR
                    attn = a2.tile([P, HPC, T], bf16)
                    for b in range(B):
                        for qt in range(QT):
                            tt = b * QT + qt
                            qc0 = b * S + qt * TW
                            nkt = 4 * qt + 4
                            for h in range(HPC):
                                dnp = aps.tile([1, TW], f32, tag="den", bufs=2, name="dnp")
                                atp = aps.tile([P, TW], f32, tag="att", bufs=2, name="atp")
                                exs = [None] * nkt

                                def consume(kt):
                                    nc.tensor.matmul(dnp[:], ones_col[:], exs[kt][:],
                                                     start=(kt == 0),
                                                     stop=(kt == nkt - 1))
                                    nc.tensor.matmul(atp[:],
                                                     vnat[:, b * KT + kt,
                                                          h * DV:(h + 1) * DV],
                                                     exs[kt][:],
                                                     start=(kt == 0),
                                                     stop=(kt == nkt - 1))

                                for kt in range(nkt):
                                    kc0 = b * S + kt * P
                                    scp = aps.tile([P, TW], f32, tag="big", bufs=3, name="scp")
                                    nc.tensor.matmul(scp[:],
                                                     knope[:, h, kc0:kc0 + P],
                                                     qsb[:, h, qc0:qc0 + TW],
                                                     start=True, stop=False)
                                    nc.tensor.matmul(scp[:],
                                                     krope[:, kc0:kc0 + P],
                                                     qrope[h][:, qc0:qc0 + TW],
                                                     start=False, stop=True)
                                    ex = awrk.tile([P, TW], bf16, tag="ex", bufs=4,
                                                   name="ex")
                                    j = kt - 4 * qt
                                    if j >= 0:
                                        mtmp = awrk.tile([P, TW], f32, tag="mt",
                                                         name="mtmp")
                                        nc.vector.tensor_tensor(mtmp[:], scp[:],
                                                                msk[:, j, :], ADD)
                                        nc.scalar.activation(ex[:], mtmp[:], AF.Exp)
                                    else:
                                        nc.scalar.activation(ex[:], scp[:], AF.Exp)
                                    exs[kt] = ex
                                    if kt >= 2:
                                        consume(kt - 2)
                                consume(max(nkt - 2, 0))
                                if nkt > 1:
                                    consume(nkt - 1)
                                # 1/denom: broadcast first, then wide reciprocal
                                drow = arow.tile([1, TW], bf16, tag="srow", name="drow")
                                nc.scalar.copy(drow[:], dnp[:])
                                dbp = aps.tile([P, TW], f32, tag="big", bufs=3, name="dbp")
                                nc.tensor.matmul(dbp[:], ones_row[:], drow[:],
                                                 start=True, stop=True)
                                dbc = awrk.tile([P, TW], bf16, tag="mt", name="dbc")
                                with nc.allow_low_precision(reason="softmax denom"):
                                    nc.vector.reciprocal(dbc[:], dbp[:])
                                nc.vector.tensor_tensor(
                                    attn[:, h, qc0:qc0 + TW], atp[:], dbc[:], MUL)
                            # o_proj partial for this token chunk
                            for ho in range(HCH):
                                op = aps.tile([P, TW], f32, tag="big", bufs=3, name="op")
                                for h in range(HPC):
                                    nc.tensor.matmul(op[:],
                                                     wo_sb[:, h, ho * P:(ho + 1) * P],
                                                     attn[:, h, qc0:qc0 + TW],
                                                     start=(h == 0),
                                                     stop=(h == HPC - 1))
                                osb = awrk.tile([P, TW], bf16, tag="ex", bufs=4, name="osb")
                                if ho % 2 == 0:
                                    nc.scalar.copy(osb[:], op[:])
                                else:
                                    nc.vector.tensor_copy(out=osb[:], in_=op[:])
                                nc.sync.dma_start(ar_in[tt][ho * P:(ho + 1) * P, :],
                                                  osb[:])
                            nc.gpsimd.collective_compute(
                                "AllReduce", ADD, ins=[ar_in[tt][:].opt()],
                                outs=[ar_out[tt][:].opt()], replica_groups=rg)

            # ============ Phase B: residual + norm + MLP ============
            with tc.tile_pool(name="bwork", bufs=1) as bbig2, \
                 tc.tile_pool(name="bwrk", bufs=3) as bwrk, \
                 tc.tile_pool(name="brow", bufs=2) as brow, \
                 tc.tile_pool(name="bps", bufs=1, space="PSUM") as bps:

                wd_sb = bbig2.tile([P, FPC // P, HID], bf16)  # 16K
                for o in range(FPC // P):
                    nc.sync.dma_start(wd_sb[:, o, :], wd.ap()[o * P:(o + 1) * P, :])

                for t in range(TT):
                    tsl = slice(t * TW, (t + 1) * TW)
                    # x = hidden + attn_out
                    x = bbig2.tile([P, HCH, TW], bf16, name="x", tag="x", bufs=2)
                    ssp3 = bps.tile([1, TW], f32, tag="ss", bufs=1, name="ssp3")
                    for o in range(HCH):
                        harp = bwrk.tile([P, TW], bf16, tag="harp", bufs=3, name="harp")
                        nc.sync.dma_start(harp[:],
                                          htb.ap()[o * P:(o + 1) * P, tsl])
                        arsb = bwrk.tile([P, TW], bf16, tag="arsb", bufs=3, name="arsb")
                        nc.sync.dma_start(arsb[:], ar_out[t][o * P:(o + 1) * P, :])
                        nc.vector.tensor_tensor(x[:, o, :], harp[:], arsb[:], ADD)
                        sq3 = bwrk.tile([P, TW], bf16, tag="sq3", bufs=2, name="sq3")
                        if o % 2 == 0:
                            nc.scalar.square(sq3[:], x[:, o, :])
                        else:
                            nc.vector.tensor_tensor(sq3[:], x[:, o, :], x[:, o, :], MUL)
                        nc.tensor.matmul(ssp3[:], ones_col[:], sq3[:],
                                         start=(o == 0), stop=(o == HCH - 1))
                    srow3 = brow.tile([1, TW], bf16, tag="srow3", name="srow3")
                    nc.scalar.activation(srow3[:], ssp3[:], AF.Sqrt,
                                         bias=epsb[:], scale=1.0 / HID)
                    bcp3 = bps.tile([P, TW], f32, tag="gu", bufs=4, name="bcp3")
                    nc.tensor.matmul(bcp3[:], ones_row[:], srow3[:], start=True, stop=True)
                    bc3 = bwrk.tile([P, TW], bf16, tag="bc3", bufs=1, name="bc3")
                    with nc.allow_low_precision(reason="rms scale bf16"):
                        nc.vector.reciprocal(bc3[:], bcp3[:])
                    h2 = bbig2.tile([P, HCH, TW], bf16, name="h2", tag="h2", bufs=2)
                    for o in range(HCH):
                        nc.vector.tensor_tensor(h2[:, o, :], x[:, o, :], bc3[:], MUL)

                    # gate/up/silu
                    act = bbig2.tile([P, FPC // P, TW], bf16, name="act", tag="act",
                                     bufs=1)
                    for fi in range(FPC // P):
                        gp = bps.tile([P, TW], f32, tag="gu", bufs=4, name="gp")
                        for o in range(HCH):
                            nc.tensor.matmul(gp[:], wg_sb[:, o, fi * P:(fi + 1) * P],
                                             h2[:, o, :],
                                             start=(o == 0), stop=(o == HCH - 1))
                        up = bps.tile([P, TW], f32, tag="gu", bufs=4, name="up")
                        for o in range(HCH):
                            nc.tensor.matmul(up[:], wu_sb[:, o, fi * P:(fi + 1) * P],
                                             h2[:, o, :],
                                             start=(o == 0), stop=(o == HCH - 1))
                        gs = bwrk.tile([P, TW], bf16, tag="gs", bufs=2, name="gs")
                        nc.scalar.activation(gs[:], gp[:], AF.Silu)
                        us = bwrk.tile([P, TW], bf16, tag="us", bufs=2, name="us")
                        nc.scalar.copy(us[:], up[:])
                        nc.vector.tensor_tensor(act[:, fi, :], us[:], gs[:], MUL)

                    # down projection partial (+x/8 fused) + RS
                    for ho in range(HCH):
                        dpp = bps.tile([P, TW], f32, tag="d", bufs=2, name="dpp")
                        for c in range(FPC // P):
                            nc.tensor.matmul(dpp[:], wd_sb[:, c, ho * P:(ho + 1) * P],
                                             act[:, c, :],
                                             start=(c == 0), stop=(c == FPC // P - 1))
                        dsb = bwrk.tile([P, TW], bf16, tag="dsb", bufs=3, name="dsb")
                        nc.vector.scalar_tensor_tensor(dsb[:], x[:, ho, :], 0.125,
                                                       dpp[:], MUL, ADD)
                        nc.sync.dma_start(rs_in[t][ho * P:(ho + 1) * P, :], dsb[:])
                    nc.gpsimd.collective_compute(
                        "ReduceScatter", ADD, ins=[rs_in[t][:].opt()],
                        outs=[rs_out[t][:].opt()], replica_groups=rg)
                    for o in range(HID // NC_N // P):
                        fin = bwrk.tile([P, TW], bf16, tag="fin", bufs=2, name="fin")
                        nc.sync.dma_start(fin[:], rs_out[t][o * P:(o + 1) * P, :])
                        finf = bwrk.tile([P, TW], f32, tag="finf", bufs=2, name="finf")
                        nc.vector.tensor_copy(out=finf[:], in_=fin[:])
                        nc.sync.dma_start(
                            out.ap()[o * P:(o + 1) * P, t * TW:(t + 1) * TW], finf[:])
                bbig.release()
    nc.compile()
    return nc


def _prep(hidden_states, positions, w_in_ln, w_q, w_kv_a, w_kv_a_ln,
          w_kv_b, w_o, w_post_ln, w_gate, w_up, w_down):
    hT = np.ascontiguousarray(
        np.asarray(hidden_states, np.float32).reshape(T, HID).T)

    pos = np.asarray(positions).reshape(-1).astype(np.float64)
    inv = ROPE_BASE ** (-np.arange(0, DR, 2, dtype=np.float64) / DR)
    fr = pos[:, None] * inv[None, :]                      # [T, 32]
    c32 = np.cos(fr).T.astype(np.float32)                 # [32, T]
    s32 = np.sin(fr).T.astype(np.float32)
    cosf = np.concatenate([c32] * 4, 0)
    sinf = np.concatenate([-s32, s32, -s32, s32], 0)

    r = np.arange(P)[:, None]
    c = np.arange(TW)[None, :]
    masks = np.stack([np.where(c >= r + j * P, 0.0, NEG) for j in range(4)],
                     1).astype(np.float32)                # [128, 4, 512]

    w_in_ln = np.asarray(w_in_ln, np.float32)
    wqf = (np.asarray(w_q, np.float32) * w_in_ln[:, None] * SCALING
           ).reshape(HID, H, DQK)
    wkvaf = np.asarray(w_kv_a, np.float32) * w_in_ln[:, None]
    kpe_w = wkvaf[:, KVR:]
    wkva_p = np.concatenate([wkvaf[:, :KVR], kpe_w[:, 0::2], kpe_w[:, 1::2]], 1)
    wkvbf = (np.asarray(w_kv_b, np.float32)
             * np.asarray(w_kv_a_ln, np.float32)[:, None]).reshape(KVR, H, DN + DV)
    w_post_ln = np.asarray(w_post_ln, np.float32)
    wgf = np.asarray(w_gate, np.float32) * w_post_ln[:, None]
    wuf = np.asarray(w_up, np.float32) * w_post_ln[:, None]
    wdf = np.asarray(w_down, np.float32)
    wof = np.asarray(w_o, np.float32).reshape(H, DV, HID)

    in_maps = []
    for core in range(NC_N):
        hs = [2 * core, 2 * core + 1]
        nopes = np.concatenate([wqf[:, h, :DN] for h in hs], 1)
        pes = []
        for h in hs:
            pe = wqf[:, h, DN:]
            pes += [pe[:, 0::2], pe[:, 1::2]]
        wq_c = np.concatenate([nopes] + pes, 1)
        in_maps.append({
            "htb": hT.astype(BF),
            "wq": wq_c.astype(BF),
            "wkva": wkva_p.astype(BF),
            "wkvbn": np.concatenate([wkvbf[:, h, :DN] for h in hs], 1).astype(BF),
            "wkvbv": np.concatenate([wkvbf[:, h, DN:] for h in hs], 1).astype(BF),
            "wo": np.concatenate([wof[h] for h in hs], 0).astype(BF),
            "wg": wgf[:, core * FPC:(core + 1) * FPC].astype(BF),
            "wu": wuf[:, core * FPC:(core + 1) * FPC].astype(BF),
            "wd": wdf[core * FPC:(core + 1) * FPC, :].astype(BF),
            "cosf": cosf.astype(BF),
            "sinf": sinf.astype(BF),
            "masks": masks,
        })
    return in_maps


def kernel(**inputs):
    if "nc" not in _CACHE:
        _CACHE["nc"] = _build()
    nc = _CACHE["nc"]
    in_maps = _prep(**inputs)
    res = run_bass_kernel_spmd(nc, in_maps, core_ids=list(range(NC_N)))
    outT = np.concatenate([res.results[c]["o"] for c in range(NC_N)], 0)
    return np.ascontiguousarray(outT.T).reshape(B, S, HID).astype(np.float32)


# revision 11
# speedup vs baseline: 1.1312x; 1.1061x over previous
"""DeepseekV2 decoder layer on 8 TRN2 NeuronCores (Bass/Tile) — v4.

Phase A is one per-token-chunk pipeline: each 512-token chunk runs
input-norm -> q proj (+rope) -> kv_a -> kv_b, and the attention group whose
K/V span is complete is emitted right after, so projection matmuls of chunk
t+1 fill attention-group t's dependency stalls.

The o_proj AllReduce (mesh-AR moves ~28MB of DMA per rank per 2MB chunk and
starves every overlapped DMA) is replaced by: ReduceScatter of o_proj
partials over HID row-blocks -> local residual add on the core's 256-row
block -> 2KB AllReduce of the per-core sum-of-squares partials -> local
normalize -> AllGather of the normed h2.  ~3.5x less collective DMA and the
residual/norm elementwise work drops 8x.  The final down_proj residual is
added locally after the second ReduceScatter (no x/8 trick needed).
"""

import numpy as np
import ml_dtypes

import concourse.bass as bass
import concourse.mybir as mybir
import concourse.tile as tile
from concourse import bacc
from concourse.bass_utils import run_bass_kernel_spmd

BF = ml_dtypes.bfloat16

B, S, HID = 2, 1024, 2048
T = B * S                      # 2048 tokens
H = 16
DN, DR = 128, 64
DQK = DN + DR
DV = 128
KVR = 512
INTER = 8192
EPS = 1e-6
ROPE_BASE = 10000.0
SCALING = DQK ** -0.5

NC_N = 8
HPC = H // NC_N                # 2 heads per core
FPC = INTER // NC_N            # 1024 inter per core
HPR = HID // NC_N              # 256 hid rows owned per core
P = 128
HCH = HID // P                 # 16 hid chunks
TT = 4                         # token chunks of 512
TW = T // TT                   # 512
KT = S // P                    # 8 k-tiles of 128 per batch
QT = S // TW                   # 2 q-chunks of 512 per batch
NEG = -30000.0

f32 = mybir.dt.float32
bf16 = mybir.dt.bfloat16
ADD = mybir.AluOpType.add
MUL = mybir.AluOpType.mult
AF = mybir.ActivationFunctionType

_CACHE = {}


def _build():
    nc = bacc.Bacc("TRN2", target_bir_lowering=False, debug=False, num_devices=NC_N)
    dp = lambda n, sh, dt: nc.dram_tensor(n, sh, dt, kind="ExternalInput")
    htb = dp("htb", [HID, T], bf16)
    hres = dp("hres", [HPR, T], bf16)           # this core's residual row-block
    wq = dp("wq", [HID, HPC * DQK], bf16)       # [h0n,h1n,h0x1,h0x2,h1x1,h1x2]
    wkva = dp("wkva", [HID, KVR + DR], bf16)
    wkvbn = dp("wkvbn", [KVR, HPC * DN], bf16)
    wkvbv = dp("wkvbv", [KVR, HPC * DV], bf16)
    wo = dp("wo", [HPC * DV, HID], bf16)
    wg = dp("wg", [HID, FPC], bf16)
    wu = dp("wu", [HID, FPC], bf16)
    wd = dp("wd", [FPC, HID], bf16)
    cosf = dp("cosf", [DR, T], bf16)
    sinf = dp("sinf", [DR, T], bf16)
    masks = dp("masks", [P, 4, TW], bf16)
    out = nc.dram_tensor("o", [HPR, T], bf16, kind="ExternalOutput")
    rg = [list(range(NC_N))]

    with tile.TileContext(nc) as tc:
        with tc.tile_pool(name="const", bufs=1) as cpool, \
             tc.tile_pool(name="dram", bufs=1, space="DRAM") as dram, \
             tc.tile_pool(name="bfront", bufs=1) as bfr:
            ones_col = cpool.tile([P, 1], bf16)
            nc.vector.memset(ones_col[:], 1.0)
            ones_row = cpool.tile([1, P], bf16)
            nc.vector.memset(ones_row[:], 1.0)
            epsb = cpool.tile([1, 1], f32)
            nc.vector.memset(epsb[:], EPS)

            rs1_in = [dram.tile([HID, TW], bf16, name=f"rs1i{t}") for t in range(TT)]
            rs1_out = [dram.tile([HPR, TW], bf16, name=f"rs1o{t}") for t in range(TT)]
            ssq_in = [dram.tile([1, TW], f32, name=f"ssqi{t}") for t in range(TT)]
            ssq_out = [dram.tile([1, TW], f32, addr_space="Shared", name=f"ssqo{t}")
                       for t in range(TT)]
            ag_in = [dram.tile([HPR, TW], bf16, name=f"agi{t}") for t in range(TT)]
            ag_out = [dram.tile([HID, TW], bf16, addr_space="Shared", name=f"ago{t}")
                      for t in range(TT)]
            rs2_in = [dram.tile([HID, TW], bf16, name=f"rs2i{t}") for t in range(TT)]
            rs2_out = [dram.tile([HPR, TW], bf16, name=f"rs2o{t}") for t in range(TT)]

            bbig_box = [None]
            w_sb_box = [None, None]

            # ============ Phase A: per-chunk projections + attention ============
            with tc.tile_pool(name="akeep", bufs=1) as akeep, \
                 tc.tile_pool(name="awrk", bufs=3) as awrk, \
                 tc.tile_pool(name="arow", bufs=2) as arow, \
                 tc.tile_pool(name="aps", bufs=1, space="PSUM") as aps:

                qsb = akeep.tile([P, HPC, T], bf16)        # 8K  nope planes
                lat = akeep.tile([P, KVR // P, T], bf16)   # 16K (kva in place)
                kpe = akeep.tile([DR, T], bf16)            # 4K
                knope = akeep.tile([P, HPC, T], bf16)      # 8K
                vnat = akeep.tile([P, T // P, HPC * DV], bf16)  # 8K
                attn = akeep.tile([P, HPC, T], bf16)       # 8K
                qrope = [akeep.tile([DR, T], bf16, name=f"qr{h}") for h in range(HPC)]
                krope = akeep.tile([DR, T], bf16)
                cs = akeep.tile([DR, T], bf16)
                nc.sync.dma_start(cs[:], cosf.ap())
                sn = akeep.tile([DR, T], bf16)
                nc.sync.dma_start(sn[:], sinf.ap())
                msk = akeep.tile([P, 4, TW], bf16)
                nc.sync.dma_start(msk[:], masks.ap())
                wkvbn_sb = akeep.tile([P, KVR // P, HPC * DN], bf16)
                for o in range(KVR // P):
                    nc.sync.dma_start(wkvbn_sb[:, o, :],
                                      wkvbn.ap()[o * P:(o + 1) * P, :])
                wkvbv_sb = akeep.tile([P, KVR // P, HPC * DV], bf16)
                for o in range(KVR // P):
                    nc.sync.dma_start(wkvbv_sb[:, o, :],
                                      wkvbv.ap()[o * P:(o + 1) * P, :])
                wo_sb = akeep.tile([P, HPC, HID], bf16)
                for h in range(HPC):
                    nc.sync.dma_start(wo_sb[:, h, :], wo.ap()[h * P:(h + 1) * P, :])

                aE = tc.alloc_tile_pool(name="aEarly", bufs=1)
                wq_sb = aE.tile([P, HCH, HPC * DQK], bf16)   # 12K
                for o in range(HCH):
                    nc.sync.dma_start(wq_sb[:, o, :], wq.ap()[o * P:(o + 1) * P, :])
                wkva_sb = aE.tile([P, HCH, KVR + DR], bf16)  # 18K
                for o in range(HCH):
                    nc.sync.dma_start(wkva_sb[:, o, :],
                                      wkva.ap()[o * P:(o + 1) * P, :])

                xs = []          # per-chunk [P, 2, TW] residual blocks (live into B)

                def attn_group(b, qt):
                    tt = b * QT + qt
                    qc0 = b * S + qt * TW
                    nkt = 4 * qt + 4
                    for h in range(HPC):
                        dnp = aps.tile([1, TW], f32, tag="row", bufs=2, name="dnp")
                        atp = aps.tile([P, TW], f32, tag="att", bufs=2, name="atp")
                        exs = [None] * nkt

                        def consume(kt):
                            nc.tensor.matmul(dnp[:], ones_col[:], exs[kt][:],
                                             start=(kt == 0), stop=(kt == nkt - 1))
                            nc.tensor.matmul(atp[:],
                                             vnat[:, b * KT + kt,
                                                  h * DV:(h + 1) * DV],
                                             exs[kt][:],
                                             start=(kt == 0), stop=(kt == nkt - 1))

                        for kt in range(nkt):
                            kc0 = b * S + kt * P
                            scp = aps.tile([P, TW], f32, tag="big", bufs=3,
                                           name="scp")
                            nc.tensor.matmul(scp[:], knope[:, h, kc0:kc0 + P],
                                             qsb[:, h, qc0:qc0 + TW],
                                             start=True, stop=False)
                            nc.tensor.matmul(scp[:], krope[:, kc0:kc0 + P],
                                             qrope[h][:, qc0:qc0 + TW],
                                             start=False, stop=True)
                            ex = awrk.tile([P, TW], bf16, tag="ex", bufs=4, name="ex")
                            j = kt - 4 * qt
                            if j >= 0:
                                mtmp = awrk.tile([P, TW], f32, tag="mt", name="mtmp")
                                nc.vector.tensor_tensor(mtmp[:], scp[:],
                                                        msk[:, j, :], ADD)
                                nc.scalar.activation(ex[:], mtmp[:], AF.Exp)
                            else:
                                nc.scalar.activation(ex[:], scp[:], AF.Exp)
                            exs[kt] = ex
                            if kt >= 2:
                                consume(kt - 2)
                        consume(max(nkt - 2, 0))
                        if nkt > 1:
                            consume(nkt - 1)
                        # 1/denom: broadcast first, then wide reciprocal
                        drow = arow.tile([1, TW], bf16, tag="drow", name="drow")
                        nc.scalar.copy(drow[:], dnp[:])
                        dbp = aps.tile([P, TW], f32, tag="big", bufs=3, name="dbp")
                        nc.tensor.matmul(dbp[:], ones_row[:], drow[:],
                                         start=True, stop=True)
                        dbc = awrk.tile([P, TW], bf16, tag="mt", name="dbc")
                        with nc.allow_low_precision(reason="softmax denom"):
                            nc.vector.reciprocal(dbc[:], dbp[:])
                        nc.vector.tensor_tensor(attn[:, h, qc0:qc0 + TW],
                                                atp[:], dbc[:], MUL)
                    # o_proj partial for this token chunk
                    for ho in range(HCH):
                        op = aps.tile([P, TW], f32, tag="big", bufs=3, name="op")
                        for h in range(HPC):
                            nc.tensor.matmul(op[:], wo_sb[:, h, ho * P:(ho + 1) * P],
                                             attn[:, h, qc0:qc0 + TW],
                                             start=(h == 0), stop=(h == HPC - 1))
                        osb = awrk.tile([P, TW], bf16, tag="ex", bufs=4, name="osb")
                        if ho % 2 == 0:
                            nc.scalar.copy(osb[:], op[:])
                        else:
                            nc.vector.tensor_copy(out=osb[:], in_=op[:])
                        nc.sync.dma_start(rs1_in[tt][ho * P:(ho + 1) * P, :], osb[:])
                    nc.gpsimd.collective_compute(
                        "ReduceScatter", ADD, ins=[rs1_in[tt][:].opt()],
                        outs=[rs1_out[tt][:].opt()], replica_groups=rg)

                def b_front1(t):
                    """residual add + local sum-of-squares partial + tiny AR."""
                    tsl = slice(t * TW, (t + 1) * TW)
                    x = bfr.tile([P, 2, TW], bf16, tag="x", bufs=4, name=f"x{t}")
                    xs.append(x)
                    sspf = aps.tile([1, TW], f32, tag="row", bufs=2, name="sspf")
                    for j in range(2):
                        rsb = awrk.tile([P, TW], bf16, tag="fr", name="rsb")
                        nc.sync.dma_start(rsb[:], rs1_out[t][j * P:(j + 1) * P, :])
                        hb = awrk.tile([P, TW], bf16, tag="fr", name="hb")
                        nc.sync.dma_start(hb[:], hres.ap()[j * P:(j + 1) * P, tsl])
                        nc.vector.tensor_tensor(x[:, j, :], rsb[:], hb[:], ADD)
                        sqf = awrk.tile([P, TW], bf16, tag="fr", name="sqf")
                        nc.scalar.square(sqf[:], x[:, j, :])
                        nc.tensor.matmul(sspf[:], ones_col[:], sqf[:],
                                         start=(j == 0), stop=(j == 1))
                    ssq_sb = arow.tile([1, TW], f32, tag="ssq", name="ssq_sb")
                    nc.vector.tensor_copy(out=ssq_sb[:], in_=sspf[:])
                    nc.sync.dma_start(ssq_in[t][:, :], ssq_sb[:])
                    nc.gpsimd.collective_compute(
                        "AllReduce", ADD, ins=[ssq_in[t][:].opt()],
                        outs=[ssq_out[t][:].opt()], replica_groups=rg)

                def b_front2(t):
                    """normalize the block with the global rms, AllGather h2."""
                    ssq_tot = arow.tile([1, TW], f32, tag="ssq", name="ssq_tot")
                    nc.sync.dma_start(ssq_tot[:], ssq_out[t][:, :])
                    srowf = arow.tile([1, TW], bf16, tag="srowf", name="srowf")
                    nc.scalar.activation(srowf[:], ssq_tot[:], AF.Sqrt,
                                         bias=epsb[:], scale=1.0 / HID)
                    bcpf = aps.tile([P, TW], f32, tag="big", bufs=3, name="bcpf")
                    nc.tensor.matmul(bcpf[:], ones_row[:], srowf[:],
                                     start=True, stop=True)
                    bcf = awrk.tile([P, TW], bf16, tag="fr", name="bcf")
                    with nc.allow_low_precision(reason="rms scale bf16"):
                        nc.vector.reciprocal(bcf[:], bcpf[:])
                    for j in range(2):
                        h2b = awrk.tile([P, TW], bf16, tag="fr", name="h2b")
                        nc.vector.tensor_tensor(h2b[:], xs[t][:, j, :], bcf[:], MUL)
                        nc.sync.dma_start(ag_in[t][j * P:(j + 1) * P, :], h2b[:])
                    nc.gpsimd.collective_compute(
                        "AllGather", mybir.AluOpType.bypass,
                        ins=[ag_in[t][:].opt()], outs=[ag_out[t][:].opt()],
                        replica_groups=rg)

                for t in range(TT):
                    tsl = slice(t * TW, (t + 1) * TW)
                    ht = aE.tile([P, HCH, TW], bf16, tag="ht", bufs=2, name=f"ht{t}")
                    for o in range(HCH):
                        nc.sync.dma_start(ht[:, o, :],
                                          htb.ap()[o * P:(o + 1) * P, tsl])

                    # ---- input rmsnorm scale ----
                    ssp = aps.tile([1, TW], f32, tag="row", bufs=2, name="ssp")
                    for o in range(HCH):
                        sq = awrk.tile([P, TW], bf16, tag="sq", name="sq")
                        if o % 2 == 0:
                            nc.scalar.square(sq[:], ht[:, o, :])
                        else:
                            nc.vector.tensor_tensor(sq[:], ht[:, o, :],
                                                    ht[:, o, :], MUL)
                        nc.tensor.matmul(ssp[:], ones_col[:], sq[:],
                                         start=(o == 0), stop=(o == HCH - 1))
                    srow = arow.tile([1, TW], bf16, tag="srow", name="srow")
                    nc.scalar.activation(srow[:], ssp[:], AF.Sqrt,
                                         bias=epsb[:], scale=1.0 / HID)
                    bcp = aps.tile([P, TW], f32, tag="big", bufs=3, name="bcp")
                    nc.tensor.matmul(bcp[:], ones_row[:], srow[:],
                                     start=True, stop=True)
                    bc1t = awrk.tile([P, TW], bf16, tag="bc", name="bc1t")
                    with nc.allow_low_precision(reason="rms scale bf16"):
                        nc.vector.reciprocal(bc1t[:], bcp[:])

                    # ---- q projection (scaled by r1; SCALING folded into wq) ----
                    qpe_t = awrk.tile([P, TW], bf16, tag="qpe", bufs=2, name="qpe_t")
                    for f in range(3):
                        qp = aps.tile([P, TW], f32, tag="big", bufs=3, name="qp")
                        for o in range(HCH):
                            nc.tensor.matmul(qp[:], wq_sb[:, o, f * P:(f + 1) * P],
                                             ht[:, o, :],
                                             start=(o == 0), stop=(o == HCH - 1))
                        dst = qsb[:, f, tsl] if f < HPC else qpe_t[:]
                        nc.vector.tensor_tensor(dst, qp[:], bc1t[:], MUL)

                    # ---- q rope for this chunk ----
                    for h in range(HPC):
                        if h == 0:
                            direct = qpe_t[0:DR, :]
                        else:
                            dcp = awrk.tile([DR, TW], bf16, tag="rope", name="dcp")
                            nc.sync.dma_start(dcp[:], qpe_t[DR:2 * DR, :])
                            direct = dcp[:]
                        sw = awrk.tile([DR, TW], bf16, tag="rope", name="qsw")
                        nc.sync.dma_start(sw[0:32, :],
                                          qpe_t[h * DR + 32:h * DR + 64, :])
                        nc.sync.dma_start(sw[32:64, :],
                                          qpe_t[h * DR:h * DR + 32, :])
                        tmp = awrk.tile([DR, TW], bf16, tag="rope", name="qtmp")
                        nc.vector.tensor_tensor(tmp[:], direct, cs[:, tsl], MUL)
                        nc.vector.tensor_tensor(qrope[h][:, tsl], sw[:],
                                                sn[:, tsl], MUL)
                        nc.vector.tensor_tensor(qrope[h][:, tsl], qrope[h][:, tsl],
                                                tmp[:], ADD)

                    # ---- kv_a: latent raw + k_pe*r1 ----
                    ss2p = aps.tile([1, TW], f32, tag="row", bufs=2, name="ss2p")
                    for f in range(KVR // P + 1):
                        wid = P if f < KVR // P else DR
                        lp = aps.tile([P, TW], f32, tag="big", bufs=3, name="lp")
                        for o in range(HCH):
                            nc.tensor.matmul(lp[:wid, :],
                                             wkva_sb[:, o, f * P:f * P + wid],
                                             ht[:, o, :],
                                             start=(o == 0), stop=(o == HCH - 1))
                        if f < KVR // P:
                            nc.vector.tensor_copy(out=lat[:, f, tsl], in_=lp[:])
                            sq2 = awrk.tile([P, TW], bf16, tag="sq", name="sq2")
                            nc.scalar.square(sq2[:], lp[:])
                            nc.tensor.matmul(ss2p[:], ones_col[:], sq2[:],
                                             start=(f == 0),
                                             stop=(f == KVR // P - 1))
                        else:
                            nc.vector.tensor_tensor(kpe[:, tsl], lp[:DR, :],
                                                    bc1t[:DR, :], MUL)
                    srow2 = arow.tile([1, TW], bf16, tag="srow", name="srow2")
                    nc.scalar.activation(srow2[:], ss2p[:], AF.Sqrt,
                                         bias=epsb[:], scale=1.0 / KVR)
                    bcp2 = aps.tile([P, TW], f32, tag="big", bufs=3, name="bcp2")
                    nc.tensor.matmul(bcp2[:], ones_row[:], srow2[:],
                                     start=True, stop=True)
                    bc2t = awrk.tile([P, TW], bf16, tag="bc", name="bc2t")
                    with nc.allow_low_precision(reason="rms scale bf16"):
                        nc.vector.reciprocal(bc2t[:], bcp2[:])
                    for f in range(KVR // P):
                        nc.vector.tensor_tensor(lat[:, f, tsl], lat[:, f, tsl],
                                                bc2t[:], MUL)

                    # ---- k rope for this chunk ----
                    ksw = awrk.tile([DR, TW], bf16, tag="rope", name="ksw")
                    nc.sync.dma_start(ksw[0:32, :], kpe[32:64, tsl])
                    nc.sync.dma_start(ksw[32:64, :], kpe[0:32, tsl])
                    ktmp = awrk.tile([DR, TW], bf16, tag="rope", name="ktmp")
                    nc.vector.tensor_tensor(ktmp[:], kpe[:, tsl], cs[:, tsl], MUL)
                    nc.vector.tensor_tensor(krope[:, tsl], ksw[:], sn[:, tsl], MUL)
                    nc.vector.tensor_tensor(krope[:, tsl], krope[:, tsl],
                                            ktmp[:], ADD)

                    # ---- kv_b for this chunk ----
                    for h in range(HPC):
                        kp = aps.tile([P, TW], f32, tag="big", bufs=3, name="kp")
                        for c in range(KVR // P):
                            nc.tensor.matmul(kp[:], wkvbn_sb[:, c, h * P:(h + 1) * P],
                                             lat[:, c, tsl],
                                             start=(c == 0), stop=(c == KVR // P - 1))
                        if h == 0:
                            nc.scalar.copy(knope[:, h, tsl], kp[:])
                        else:
                            nc.vector.tensor_copy(out=knope[:, h, tsl], in_=kp[:])
                    for to in range(4 * t, 4 * t + 4):
                        vp = aps.tile([P, HPC * DV], f32, tag="vp", bufs=1, name="vp")
                        for c in range(KVR // P):
                            nc.tensor.matmul(vp[:], lat[:, c, to * P:(to + 1) * P],
                                             wkvbv_sb[:, c, :],
                                             start=(c == 0), stop=(c == KVR // P - 1))
                        nc.scalar.copy(vnat[:, to, :], vp[:])

                    # ---- lagged b-front stages (avoid DMA-queue head blocking) ----
                    if t >= 1:
                        b_front1(t - 1)
                    if t >= 2:
                        b_front2(t - 2)
                    # ---- attention group whose K/V span is now complete ----
                    if t == 3:
                        aE.release()
                        bbig = tc.alloc_tile_pool(name="bbig", bufs=1,
                                                  side="right")
                        bbig_box[0] = bbig
                        wg_sb = bbig.tile([P, HCH, FPC], bf16)       # 32K
                        for o in range(HCH):
                            nc.sync.dma_start(wg_sb[:, o, :],
                                              wg.ap()[o * P:(o + 1) * P, :])
                        wu_sb = bbig.tile([P, HCH, FPC], bf16)       # 32K
                        for o in range(HCH):
                            nc.sync.dma_start(wu_sb[:, o, :],
                                              wu.ap()[o * P:(o + 1) * P, :])
                        w_sb_box[0], w_sb_box[1] = wg_sb, wu_sb
                    attn_group(t // 2, t % 2)

                b_front1(3)
                b_front2(2)
                b_front2(3)

            # ============ Phase B: MLP on gathered h2 + RS2 + residual ============
            wg_sb, wu_sb = w_sb_box
            with tc.tile_pool(name="bwork", bufs=1) as bbig2, \
                 tc.tile_pool(name="bwrk", bufs=3) as bwrk, \
                 tc.tile_pool(name="bps", bufs=1, space="PSUM") as bps:

                wd_sb = bbig2.tile([P, FPC // P, HID], bf16)  # 16K
                for o in range(FPC // P):
                    nc.sync.dma_start(wd_sb[:, o, :], wd.ap()[o * P:(o + 1) * P, :])

                def b_fin(t):
                    # final residual add on this core's row block, bf16 out
                    tsl = slice(t * TW, (t + 1) * TW)
                    for j in range(2):
                        dob = bwrk.tile([P, TW], bf16, tag="dob", bufs=2, name="dob")
                        nc.sync.dma_start(dob[:], rs2_out[t][j * P:(j + 1) * P, :])
                        fo = bwrk.tile([P, TW], bf16, tag="fo", bufs=2, name="fo")
                        nc.vector.tensor_tensor(fo[:], dob[:], xs[t][:, j, :], ADD)
                        nc.sync.dma_start(out.ap()[j * P:(j + 1) * P, tsl], fo[:])

                for t in range(TT):
                    tsl = slice(t * TW, (t + 1) * TW)
                    h2 = bbig2.tile([P, HCH, TW], bf16, name="h2", tag="h2", bufs=2)
                    for o in range(HCH):
                        nc.sync.dma_start(h2[:, o, :],
                                          ag_out[t][o * P:(o + 1) * P, :])

                    # gate/up/silu
                    act = bbig2.tile([P, FPC // P, TW], bf16, name="act", tag="act",
                                     bufs=1)
                    for fi in range(FPC // P):
                        gp = bps.tile([P, TW], f32, tag="gu", bufs=4, name="gp")
                        for o in range(HCH):
                            nc.tensor.matmul(gp[:], wg_sb[:, o, fi * P:(fi + 1) * P],
                                             h2[:, o, :],
                                             start=(o == 0), stop=(o == HCH - 1))
                        up = bps.tile([P, TW], f32, tag="gu", bufs=4, name="up")
                        for o in range(HCH):
                            nc.tensor.matmul(up[:], wu_sb[:, o, fi * P:(fi + 1) * P],
                                             h2[:, o, :],
                                             start=(o == 0), stop=(o == HCH - 1))
                        gs = bwrk.tile([P, TW], bf16, tag="gs", bufs=2, name="gs")
                        nc.scalar.activation(gs[:], gp[:], AF.Silu)
                        us = bwrk.tile([P, TW], bf16, tag="us", bufs=2, name="us")
                        nc.scalar.copy(us[:], up[:])
                        nc.vector.tensor_tensor(act[:, fi, :], us[:], gs[:], MUL)

                    if t >= 1:
                        b_fin(t - 1)

                    # down projection partial + RS2
                    for ho in range(HCH):
                        dpp = bps.tile([P, TW], f32, tag="d", bufs=2, name="dpp")
                        for c in range(FPC // P):
                            nc.tensor.matmul(dpp[:], wd_sb[:, c, ho * P:(ho + 1) * P],
                                             act[:, c, :],
                                             start=(c == 0), stop=(c == FPC // P - 1))
                        dsb = bwrk.tile([P, TW], bf16, tag="dsb", bufs=3, name="dsb")
                        if ho % 2 == 0:
                            nc.scalar.copy(dsb[:], dpp[:])
                        else:
                            nc.vector.tensor_copy(out=dsb[:], in_=dpp[:])
                        nc.sync.dma_start(rs2_in[t][ho * P:(ho + 1) * P, :], dsb[:])
                    nc.gpsimd.collective_compute(
                        "ReduceScatter", ADD, ins=[rs2_in[t][:].opt()],
                        outs=[rs2_out[t][:].opt()], replica_groups=rg)
                b_fin(TT - 1)
                bbig_box[0].release()
    nc.compile()
    return nc


def _prep(hidden_states, positions, w_in_ln, w_q, w_kv_a, w_kv_a_ln,
          w_kv_b, w_o, w_post_ln, w_gate, w_up, w_down):
    hT = np.ascontiguousarray(
        np.asarray(hidden_states, np.float32).reshape(T, HID).T)

    pos = np.asarray(positions).reshape(-1).astype(np.float64)
    inv = ROPE_BASE ** (-np.arange(0, DR, 2, dtype=np.float64) / DR)
    fr = pos[:, None] * inv[None, :]                      # [T, 32]
    c32 = np.cos(fr).T.astype(np.float32)                 # [32, T]
    s32 = np.sin(fr).T.astype(np.float32)
    cosf = np.concatenate([c32, c32], 0)                  # [64, T]
    sinf = np.concatenate([-s32, s32], 0)

    r = np.arange(P)[:, None]
    c = np.arange(TW)[None, :]
    masks = np.stack([np.where(c >= r + j * P, 0.0, NEG) for j in range(4)],
                     1).astype(np.float32)                # [128, 4, 512]

    w_in_ln = np.asarray(w_in_ln, np.float32)
    wqf = (np.asarray(w_q, np.float32) * w_in_ln[:, None] * SCALING
           ).reshape(HID, H, DQK)
    wkvaf = np.asarray(w_kv_a, np.float32) * w_in_ln[:, None]
    kpe_w = wkvaf[:, KVR:]
    wkva_p = np.concatenate([wkvaf[:, :KVR], kpe_w[:, 0::2], kpe_w[:, 1::2]], 1)
    wkvbf = (np.asarray(w_kv_b, np.float32)
             * np.asarray(w_kv_a_ln, np.float32)[:, None]).reshape(KVR, H, DN + DV)
    w_post_ln = np.asarray(w_post_ln, np.float32)
    wgf = np.asarray(w_gate, np.float32) * w_post_ln[:, None]
    wuf = np.asarray(w_up, np.float32) * w_post_ln[:, None]
    wdf = np.asarray(w_down, np.float32)
    wof = np.asarray(w_o, np.float32).reshape(H, DV, HID)

    hT_bf = hT.astype(BF)
    in_maps = []
    for core in range(NC_N):
        hs = [2 * core, 2 * core + 1]
        nopes = np.concatenate([wqf[:, h, :DN] for h in hs], 1)
        pes = []
        for h in hs:
            pe = wqf[:, h, DN:]
            pes += [pe[:, 0::2], pe[:, 1::2]]
        wq_c = np.concatenate([nopes] + pes, 1)
        in_maps.append({
            "htb": hT_bf,
            "hres": np.ascontiguousarray(hT_bf[core * HPR:(core + 1) * HPR, :]),
            "wq": wq_c.astype(BF),
            "wkva": wkva_p.astype(BF),
            "wkvbn": np.concatenate([wkvbf[:, h, :DN] for h in hs], 1).astype(BF),
            "wkvbv": np.concatenate([wkvbf[:, h, DN:] for h in hs], 1).astype(BF),
            "wo": np.concatenate([wof[h] for h in hs], 0).astype(BF),
            "wg": wgf[:, core * FPC:(core + 1) * FPC].astype(BF),
            "wu": wuf[:, core * FPC:(core + 1) * FPC].astype(BF),
            "wd": wdf[core * FPC:(core + 1) * FPC, :].astype(BF),
            "cosf": cosf.astype(BF),
            "sinf": sinf.astype(BF),
            "masks": masks.astype(BF),
        })
    return in_maps


def kernel(**inputs):
    if "nc" not in _CACHE:
        _CACHE["nc"] = _build()
    nc = _CACHE["nc"]
    in_maps = _prep(**inputs)
    res = run_bass_kernel_spmd(nc, in_maps, core_ids=list(range(NC_N)))
    outT = np.concatenate([np.asarray(res.results[c]["o"], np.float32)
                           for c in range(NC_N)], 0)
    return np.ascontiguousarray(outT.T).reshape(B, S, HID).astype(np.float32)


# revision 16
# speedup vs baseline: 1.2509x; 1.1058x over previous
"""DeepseekV2 decoder layer on 8 TRN2 NeuronCores (Bass/Tile) — v4.

Phase A is one per-token-chunk pipeline: each 512-token chunk runs
input-norm -> q proj (+rope) -> kv_a -> kv_b, and the attention group whose
K/V span is complete is emitted right after, so projection matmuls of chunk
t+1 fill attention-group t's dependency stalls.

The o_proj AllReduce (mesh-AR moves ~28MB of DMA per rank per 2MB chunk and
starves every overlapped DMA) is replaced by: ReduceScatter of o_proj
partials over HID row-blocks -> local residual add on the core's 256-row
block -> 2KB AllReduce of the per-core sum-of-squares partials -> local
normalize -> AllGather of the normed h2.  ~3.5x less collective DMA and the
residual/norm elementwise work drops 8x.  The final down_proj residual is
added locally after the second ReduceScatter (no x/8 trick needed).
"""

import numpy as np
import ml_dtypes

import concourse.bass as bass
import concourse.mybir as mybir
import concourse.tile as tile
from concourse import bacc
from concourse.bass_utils import run_bass_kernel_spmd

BF = ml_dtypes.bfloat16

B, S, HID = 2, 1024, 2048
T = B * S                      # 2048 tokens
H = 16
DN, DR = 128, 64
DQK = DN + DR
DV = 128
KVR = 512
INTER = 8192
EPS = 1e-6
ROPE_BASE = 10000.0
SCALING = DQK ** -0.5

NC_N = 8
HPC = H // NC_N                # 2 heads per core
FPC = INTER // NC_N            # 1024 inter per core
HPR = HID // NC_N              # 256 hid rows owned per core
P = 128
HCH = HID // P                 # 16 hid chunks
TT = 4                         # token chunks of 512
TW = T // TT                   # 512
KT = S // P                    # 8 k-tiles of 128 per batch
QT = S // TW                   # 2 q-chunks of 512 per batch
NEG = -30000.0

f32 = mybir.dt.float32
bf16 = mybir.dt.bfloat16
ADD = mybir.AluOpType.add
MUL = mybir.AluOpType.mult
AF = mybir.ActivationFunctionType

_CACHE = {}


def _build():
    nc = bacc.Bacc("TRN2", target_bir_lowering=False, debug=False, num_devices=NC_N)
    dp = lambda n, sh, dt: nc.dram_tensor(n, sh, dt, kind="ExternalInput")
    htb = dp("htb", [HID, T], bf16)
    hres = dp("hres", [HPR, T], bf16)           # this core's residual row-block
    wq = dp("wq", [HID, HPC * DQK], bf16)       # [h0n,h1n,h0x1,h0x2,h1x1,h1x2]
    wkva = dp("wkva", [HID, KVR + DR], bf16)
    wkvbn = dp("wkvbn", [KVR, HPC * DN], bf16)
    wkvbv = dp("wkvbv", [KVR, HPC * DV], bf16)
    wo = dp("wo", [HPC * DV, HID], bf16)
    wg = dp("wg", [HID, FPC], bf16)
    wu = dp("wu", [HID, FPC], bf16)
    wd = dp("wd", [FPC, HID], bf16)
    cosf = dp("cosf", [DR, T], bf16)
    sinf = dp("sinf", [DR, T], bf16)
    masks = dp("masks", [P, 4, TW], bf16)
    out = nc.dram_tensor("o", [HPR, T], bf16, kind="ExternalOutput")
    rg = [list(range(NC_N))]

    with tile.TileContext(nc) as tc:
        with tc.tile_pool(name="const", bufs=1) as cpool, \
             tc.tile_pool(name="dram", bufs=1, space="DRAM") as dram, \
             tc.tile_pool(name="bfront", bufs=1) as bfr:
            ones_col = cpool.tile([P, 1], bf16)
            nc.vector.memset(ones_col[:], 1.0)
            ones_row = cpool.tile([1, P], bf16)
            nc.vector.memset(ones_row[:], 1.0)
            epsb = cpool.tile([1, 1], f32)
            nc.vector.memset(epsb[:], EPS)

            rs1_in = [dram.tile([HID, TW], bf16, name=f"rs1i{t}") for t in range(TT)]
            rs1_out = [dram.tile([HPR, TW], bf16, name=f"rs1o{t}") for t in range(TT)]
            ssq_in = [dram.tile([1, TW], f32, name=f"ssqi{t}") for t in range(TT)]
            ssq_out = [dram.tile([1, TW], f32, addr_space="Shared", name=f"ssqo{t}")
                       for t in range(TT)]
            ag_in = [dram.tile([HPR, TW], bf16, name=f"agi{t}") for t in range(TT)]
            ag_out = [dram.tile([HID, TW], bf16, addr_space="Shared", name=f"ago{t}")
                      for t in range(TT)]
            rs2_in = [dram.tile([HID, TW], bf16, name=f"rs2i{t}") for t in range(TT)]
            rs2_out = [dram.tile([HPR, TW], bf16, name=f"rs2o{t}") for t in range(TT)]
            rs2h_in = [dram.tile([HID // 2, TW], bf16, name=f"rs2hi{j}")
                       for j in range(2)]
            rs2h_out = [dram.tile([HPR // 2, TW], bf16, name=f"rs2ho{j}")
                        for j in range(2)]

            bbig_box = [None]
            w_sb_box = [None, None]

            # ============ Phase A: per-chunk projections + attention ============
            with tc.tile_pool(name="akeep", bufs=1) as akeep, \
                 tc.tile_pool(name="awrk", bufs=3) as awrk, \
                 tc.tile_pool(name="arow", bufs=2) as arow, \
                 tc.tile_pool(name="aps", bufs=1, space="PSUM") as aps:

                qsb = akeep.tile([P, HPC, T], bf16)        # 8K  nope planes
                lat = akeep.tile([P, KVR // P, T], bf16)   # 16K (kva in place)
                kpe = akeep.tile([DR, T], bf16)            # 4K
                knope = akeep.tile([P, HPC, T], bf16)      # 8K
                vnat = akeep.tile([P, T // P, HPC * DV], bf16)  # 8K
                attn = akeep.tile([P, HPC, T], bf16)       # 8K
                qrope = [akeep.tile([DR, T], bf16, name=f"qr{h}") for h in range(HPC)]
                krope = akeep.tile([DR, T], bf16)
                cs = akeep.tile([DR, T], bf16)
                sn = akeep.tile([DR, T], bf16)
                msk = akeep.tile([P, 4, TW], bf16)
                wkvbn_sb = akeep.tile([P, KVR // P, HPC * DN], bf16)
                wkvbv_sb = akeep.tile([P, KVR // P, HPC * DV], bf16)
                wo_sb = akeep.tile([P, HPC, HID], bf16)

                aE = tc.alloc_tile_pool(name="aEarly", bufs=1)
                wq_sb = aE.tile([P, HCH, HPC * DQK], bf16)   # 12K
                wkva_sb = aE.tile([P, HCH, KVR + DR], bf16)  # 18K

                xs = []          # per-chunk [P, 2, TW] residual blocks (live into B)

                def attn_group(b, qt):
                    tt = b * QT + qt
                    qc0 = b * S + qt * TW
                    nkt = 4 * qt + 4
                    dnps, atps = [], []
                    for h in range(HPC):
                        dnp = aps.tile([1, TW], f32, tag="row", bufs=2, name="dnp")
                        atp = aps.tile([P, TW], f32, tag="att", bufs=2, name="atp")
                        exs = [None] * nkt

                        def consume(kt, dnp=dnp, atp=atp, exs=exs, h=h):
                            nc.tensor.matmul(dnp[:], ones_col[:], exs[kt][:],
                                             start=(kt == 0), stop=(kt == nkt - 1))
                            nc.tensor.matmul(atp[:],
                                             vnat[:, b * KT + kt,
                                                  h * DV:(h + 1) * DV],
                                             exs[kt][:],
                                             start=(kt == 0), stop=(kt == nkt - 1))

                        for kt in range(nkt):
                            kc0 = b * S + kt * P
                            scp = aps.tile([P, TW], f32, tag="big", bufs=3,
                                           name="scp")
                            nc.tensor.matmul(scp[:], knope[:, h, kc0:kc0 + P],
                                             qsb[:, h, qc0:qc0 + TW],
                                             start=True, stop=False)
                            nc.tensor.matmul(scp[:], krope[:, kc0:kc0 + P],
                                             qrope[h][:, qc0:qc0 + TW],
                                             start=False, stop=True)
                            ex = awrk.tile([P, TW], bf16, tag="ex", bufs=6, name="ex")
                            j = kt - 4 * qt
                            if j >= 0:
                                mtmp = awrk.tile([P, TW], f32, tag="mt", name="mtmp")
                                nc.vector.tensor_tensor(mtmp[:], scp[:],
                                                        msk[:, j, :], ADD)
                                nc.scalar.activation(ex[:], mtmp[:], AF.Exp)
                            else:
                                nc.scalar.activation(ex[:], scp[:], AF.Exp)
                            exs[kt] = ex
                            if kt >= 2:
                                consume(kt - 2)
                        consume(max(nkt - 2, 0))
                        if nkt > 1:
                            consume(nkt - 1)
                        dnps.append(dnp)
                        atps.append(atp)
                    for h in range(HPC):
                        # 1/denom: broadcast first, then wide reciprocal
                        drow = arow.tile([1, TW], bf16, tag="drow", name="drow")
                        nc.scalar.copy(drow[:], dnps[h][:])
                        dbp = aps.tile([P, TW], f32, tag="big", bufs=3, name="dbp")
                        nc.tensor.matmul(dbp[:], ones_row[:], drow[:],
                                         start=True, stop=True)
                        dbc = awrk.tile([P, TW], bf16, tag="mt", name="dbc")
                        with nc.allow_low_precision(reason="softmax denom"):
                            nc.vector.reciprocal(dbc[:], dbp[:])
                        nc.vector.tensor_tensor(attn[:, h, qc0:qc0 + TW],
                                                atps[h][:], dbc[:], MUL)
                    # o_proj partial for this token chunk
                    for ho in range(HCH):
                        op = aps.tile([P, TW], f32, tag="big", bufs=3, name="op")
                        for h in range(HPC):
                            nc.tensor.matmul(op[:], wo_sb[:, h, ho * P:(ho + 1) * P],
                                             attn[:, h, qc0:qc0 + TW],
                                             start=(h == 0), stop=(h == HPC - 1))
                        osb = awrk.tile([P, TW], bf16, tag="ex", bufs=6, name="osb")
                        if ho % 2 == 0:
                            nc.scalar.copy(osb[:], op[:])
                        else:
                            nc.vector.tensor_copy(out=osb[:], in_=op[:])
                        nc.sync.dma_start(rs1_in[tt][ho * P:(ho + 1) * P, :], osb[:])
                    nc.gpsimd.collective_compute(
                        "ReduceScatter", ADD, ins=[rs1_in[tt][:].opt()],
                        outs=[rs1_out[tt][:].opt()], replica_groups=rg)

                def b_front1(t, wrk, row, ps, rbufs):
                    """residual add + local sum-of-squares partial + tiny AR."""
                    tsl = slice(t * TW, (t + 1) * TW)
                    x = bfr.tile([P, 2, TW], bf16, tag="x", bufs=4, name=f"x{t}")
                    xs.append(x)
                    sspf = ps.tile([1, TW], f32, tag="row", bufs=rbufs, name="sspf")
                    for j in range(2):
                        rsb = wrk.tile([P, TW], bf16, tag="fr", name="rsb")
                        nc.sync.dma_start(rsb[:], rs1_out[t][j * P:(j + 1) * P, :])
                        hb = wrk.tile([P, TW], bf16, tag="fr", name="hb")
                        nc.sync.dma_start(hb[:], hres.ap()[j * P:(j + 1) * P, tsl])
                        nc.vector.tensor_tensor(x[:, j, :], rsb[:], hb[:], ADD)
                        sqf = wrk.tile([P, TW], bf16, tag="fr", name="sqf")
                        nc.scalar.square(sqf[:], x[:, j, :])
                        nc.tensor.matmul(sspf[:], ones_col[:], sqf[:],
                                         start=(j == 0), stop=(j == 1))
                    ssq_sb = row.tile([1, TW], f32, tag="ssq", name="ssq_sb")
                    nc.vector.tensor_copy(out=ssq_sb[:], in_=sspf[:])
                    nc.sync.dma_start(ssq_in[t][:, :], ssq_sb[:])
                    nc.gpsimd.collective_compute(
                        "AllReduce", ADD, ins=[ssq_in[t][:].opt()],
                        outs=[ssq_out[t][:].opt()], replica_groups=rg)

                def b_front2(t, wrk, row, ps, big_tag, bbufs):
                    """normalize the block with the global rms, AllGather h2."""
                    ssq_tot = row.tile([1, TW], f32, tag="ssq", name="ssq_tot")
                    nc.sync.dma_start(ssq_tot[:], ssq_out[t][:, :])
                    srowf = row.tile([1, TW], bf16, tag="srowf", name="srowf")
                    nc.scalar.activation(srowf[:], ssq_tot[:], AF.Sqrt,
                                         bias=epsb[:], scale=1.0 / HID)
                    bcpf = ps.tile([P, TW], f32, tag=big_tag, bufs=bbufs, name="bcpf")
                    nc.tensor.matmul(bcpf[:], ones_row[:], srowf[:],
                                     start=True, stop=True)
                    bcf = wrk.tile([P, TW], bf16, tag="fr", name="bcf")
                    with nc.allow_low_precision(reason="rms scale bf16"):
                        nc.vector.reciprocal(bcf[:], bcpf[:])
                    for j in range(2):
                        h2b = wrk.tile([P, TW], bf16, tag="fr", name="h2b")
                        nc.vector.tensor_tensor(h2b[:], xs[t][:, j, :], bcf[:], MUL)
                        nc.sync.dma_start(ag_in[t][j * P:(j + 1) * P, :], h2b[:])
                    nc.gpsimd.collective_compute(
                        "AllGather", mybir.AluOpType.bypass,
                        ins=[ag_in[t][:].opt()], outs=[ag_out[t][:].opt()],
                        replica_groups=rg)

                for t in range(TT):
                    tsl = slice(t * TW, (t + 1) * TW)
                    ht = aE.tile([P, HCH, TW], bf16, tag="ht", bufs=2, name=f"ht{t}")
                    for o in range(HCH):
                        nc.sync.dma_start(ht[:, o, :],
                                          htb.ap()[o * P:(o + 1) * P, tsl])
                    if t == 0:
                        for o in range(HCH):
                            nc.sync.dma_start(wq_sb[:, o, :],
                                              wq.ap()[o * P:(o + 1) * P, :])
                        for o in range(HCH):
                            nc.sync.dma_start(wkva_sb[:, o, :],
                                              wkva.ap()[o * P:(o + 1) * P, :])

                    # ---- input rmsnorm scale ----
                    ssp = aps.tile([1, TW], f32, tag="row", bufs=2, name="ssp")
                    for o in range(HCH):
                        sq = awrk.tile([P, TW], bf16, tag="sq", name="sq")
                        if o % 2 == 0:
                            nc.scalar.square(sq[:], ht[:, o, :])
                        else:
                            nc.vector.tensor_tensor(sq[:], ht[:, o, :],
                                                    ht[:, o, :], MUL)
                        nc.tensor.matmul(ssp[:], ones_col[:], sq[:],
                                         start=(o == 0), stop=(o == HCH - 1))
                    srow = arow.tile([1, TW], bf16, tag="srow", name="srow")
                    nc.scalar.activation(srow[:], ssp[:], AF.Sqrt,
                                         bias=epsb[:], scale=1.0 / HID)
                    bcp = aps.tile([P, TW], f32, tag="big", bufs=3, name="bcp")
                    nc.tensor.matmul(bcp[:], ones_row[:], srow[:],
                                     start=True, stop=True)
                    bc1t = awrk.tile([P, TW], bf16, tag="bc", name="bc1t")
                    with nc.allow_low_precision(reason="rms scale bf16"):
                        nc.vector.reciprocal(bc1t[:], bcp[:])

                    # ---- q projection (scaled by r1; SCALING folded into wq) ----
                    qpe_t = awrk.tile([P, TW], bf16, tag="qpe", bufs=2, name="qpe_t")
                    for f in range(3):
                        qp = aps.tile([P, TW], f32, tag="big", bufs=3, name="qp")
                        for o in range(HCH):
                            nc.tensor.matmul(qp[:], wq_sb[:, o, f * P:(f + 1) * P],
                                             ht[:, o, :],
                                             start=(o == 0), stop=(o == HCH - 1))
                        dst = qsb[:, f, tsl] if f < HPC else qpe_t[:]
                        nc.vector.tensor_tensor(dst, qp[:], bc1t[:], MUL)

                    if t == 0:
                        nc.sync.dma_start(cs[:], cosf.ap())
                        nc.sync.dma_start(sn[:], sinf.ap())
                        nc.sync.dma_start(msk[:], masks.ap())
                        for o in range(KVR // P):
                            nc.sync.dma_start(wkvbn_sb[:, o, :],
                                              wkvbn.ap()[o * P:(o + 1) * P, :])
                        for o in range(KVR // P):
                            nc.sync.dma_start(wkvbv_sb[:, o, :],
                                              wkvbv.ap()[o * P:(o + 1) * P, :])
                        for h in range(HPC):
                            nc.sync.dma_start(wo_sb[:, h, :],
                                              wo.ap()[h * P:(h + 1) * P, :])

                    # ---- q rope for this chunk ----
                    for h in range(HPC):
                        if h == 0:
                            direct = qpe_t[0:DR, :]
                        else:
                            dcp = awrk.tile([DR, TW], bf16, tag="rope", name="dcp")
                            nc.sync.dma_start(dcp[:], qpe_t[DR:2 * DR, :])
                            direct = dcp[:]
                        sw = awrk.tile([DR, TW], bf16, tag="rope", name="qsw")
                        nc.sync.dma_start(sw[0:32, :],
                                          qpe_t[h * DR + 32:h * DR + 64, :])
                        nc.sync.dma_start(sw[32:64, :],
                                          qpe_t[h * DR:h * DR + 32, :])
                        tmp = awrk.tile([DR, TW], bf16, tag="rope", name="qtmp")
                        nc.vector.tensor_tensor(tmp[:], direct, cs[:, tsl], MUL)
                        nc.vector.tensor_tensor(qrope[h][:, tsl], sw[:],
                                                sn[:, tsl], MUL)
                        nc.vector.tensor_tensor(qrope[h][:, tsl], qrope[h][:, tsl],
                                                tmp[:], ADD)

                    # ---- kv_a: latent raw + k_pe*r1 ----
                    ss2p = aps.tile([1, TW], f32, tag="row", bufs=2, name="ss2p")
                    for f in range(KVR // P + 1):
                        wid = P if f < KVR // P else DR
                        lp = aps.tile([P, TW], f32, tag="big", bufs=3, name="lp")
                        for o in range(HCH):
                            nc.tensor.matmul(lp[:wid, :],
                                             wkva_sb[:, o, f * P:f * P + wid],
                                             ht[:, o, :],
                                             start=(o == 0), stop=(o == HCH - 1))
                        if f < KVR // P:
                            nc.vector.tensor_copy(out=lat[:, f, tsl], in_=lp[:])
                            sq2 = awrk.tile([P, TW], bf16, tag="sq", name="sq2")
                            nc.scalar.square(sq2[:], lp[:])
                            nc.tensor.matmul(ss2p[:], ones_col[:], sq2[:],
                                             start=(f == 0),
                                             stop=(f == KVR // P - 1))
                        else:
                            nc.vector.tensor_tensor(kpe[:, tsl], lp[:DR, :],
                                                    bc1t[:DR, :], MUL)
                    srow2 = arow.tile([1, TW], bf16, tag="srow", name="srow2")
                    nc.scalar.activation(srow2[:], ss2p[:], AF.Sqrt,
                                         bias=epsb[:], scale=1.0 / KVR)
                    bcp2 = aps.tile([P, TW], f32, tag="big", bufs=3, name="bcp2")
                    nc.tensor.matmul(bcp2[:], ones_row[:], srow2[:],
                                     start=True, stop=True)
                    bc2t = awrk.tile([P, TW], bf16, tag="bc", name="bc2t")
                    with nc.allow_low_precision(reason="rms scale bf16"):
                        nc.vector.reciprocal(bc2t[:], bcp2[:])
                    for f in range(KVR // P):
                        nc.vector.tensor_tensor(lat[:, f, tsl], lat[:, f, tsl],
                                                bc2t[:], MUL)

                    # ---- k rope for this chunk ----
                    ksw = awrk.tile([DR, TW], bf16, tag="rope", name="ksw")
                    nc.sync.dma_start(ksw[0:32, :], kpe[32:64, tsl])
                    nc.sync.dma_start(ksw[32:64, :], kpe[0:32, tsl])
                    ktmp = awrk.tile([DR, TW], bf16, tag="rope", name="ktmp")
                    nc.vector.tensor_tensor(ktmp[:], kpe[:, tsl], cs[:, tsl], MUL)
                    nc.vector.tensor_tensor(krope[:, tsl], ksw[:], sn[:, tsl], MUL)
                    nc.vector.tensor_tensor(krope[:, tsl], krope[:, tsl],
                                            ktmp[:], ADD)

                    # ---- kv_b for this chunk ----
                    for h in range(HPC):
                        kp = aps.tile([P, TW], f32, tag="big", bufs=3, name="kp")
                        for c in range(KVR // P):
                            nc.tensor.matmul(kp[:], wkvbn_sb[:, c, h * P:(h + 1) * P],
                                             lat[:, c, tsl],
                                             start=(c == 0), stop=(c == KVR // P - 1))
                        if h == 0:
                            nc.scalar.copy(knope[:, h, tsl], kp[:])
                        else:
                            nc.vector.tensor_copy(out=knope[:, h, tsl], in_=kp[:])
                    for to in range(4 * t, 4 * t + 4):
                        vp = aps.tile([P, HPC * DV], f32, tag="vp", bufs=1, name="vp")
                        for c in range(KVR // P):
                            nc.tensor.matmul(vp[:], lat[:, c, to * P:(to + 1) * P],
                                             wkvbv_sb[:, c, :],
                                             start=(c == 0), stop=(c == KVR // P - 1))
                        nc.scalar.copy(vnat[:, to, :], vp[:])

                    # ---- attention group whose K/V span is now complete ----
                    if t == 3:
                        aE.release()
                        bbig = tc.alloc_tile_pool(name="bbig", bufs=1,
                                                  side="right")
                        bbig_box[0] = bbig
                        wg_sb = bbig.tile([P, HCH, FPC], bf16)       # 32K
                        for o in range(HCH):
                            nc.sync.dma_start(wg_sb[:, o, :],
                                              wg.ap()[o * P:(o + 1) * P, :])
                        wu_sb = bbig.tile([P, HCH, FPC], bf16)       # 32K
                        for o in range(HCH):
                            nc.sync.dma_start(wu_sb[:, o, :],
                                              wu.ap()[o * P:(o + 1) * P, :])
                        w_sb_box[0], w_sb_box[1] = wg_sb, wu_sb
                    attn_group(t // 2, t % 2)
                    # lagged fronts: far enough behind their collectives that
                    # the in-order tensor stream never blocks on them
                    if t >= 1:
                        b_front1(t - 1, awrk, arow, aps, 2)
                    if t >= 2:
                        b_front2(t - 2, awrk, arow, aps, "big", 3)

            # ============ Phase B: MLP on gathered h2 + RS2 + residual ============
            wg_sb, wu_sb = w_sb_box
            with tc.tile_pool(name="bwork", bufs=1) as bbig2, \
                 tc.tile_pool(name="bwrk", bufs=3) as bwrk, \
                 tc.tile_pool(name="bps", bufs=1, space="PSUM") as bps:

                wd_sb = bbig2.tile([P, FPC // P, HID], bf16)  # 16K
                for o in range(FPC // P):
                    nc.sync.dma_start(wd_sb[:, o, :], wd.ap()[o * P:(o + 1) * P, :])

                def b_fin(t):
                    # final residual add on this core's row block, bf16 out
                    tsl = slice(t * TW, (t + 1) * TW)
                    for j in range(2):
                        dob = bwrk.tile([P, TW], bf16, tag="dob", bufs=2, name="dob")
                        if t == TT - 1:
                            nc.sync.dma_start(dob[:], rs2h_out[j][:, :])
                        else:
                            nc.sync.dma_start(dob[:], rs2_out[t][j * P:(j + 1) * P, :])
                        fo = bwrk.tile([P, TW], bf16, tag="fo", bufs=2, name="fo")
                        nc.vector.tensor_tensor(fo[:], dob[:], xs[t][:, j, :], ADD)
                        nc.sync.dma_start(out.ap()[j * P:(j + 1) * P, tsl], fo[:])

                for t in range(TT):
                    tsl = slice(t * TW, (t + 1) * TW)
                    h2 = bbig2.tile([P, HCH, TW], bf16, name="h2", tag="h2", bufs=2)
                    for o in range(HCH):
                        nc.sync.dma_start(h2[:, o, :],
                                          ag_out[t][o * P:(o + 1) * P, :])

                    # gate/up/silu
                    act = bbig2.tile([P, FPC // P, TW], bf16, name="act", tag="act",
                                     bufs=1)
                    for fi in range(FPC // P):
                        gp = bps.tile([P, TW], f32, tag="gu", bufs=4, name="gp")
                        for o in range(HCH):
                            nc.tensor.matmul(gp[:], wg_sb[:, o, fi * P:(fi + 1) * P],
                                             h2[:, o, :],
                                             start=(o == 0), stop=(o == HCH - 1))
                        up = bps.tile([P, TW], f32, tag="gu", bufs=4, name="up")
                        for o in range(HCH):
                            nc.tensor.matmul(up[:], wu_sb[:, o, fi * P:(fi + 1) * P],
                                             h2[:, o, :],
                                             start=(o == 0), stop=(o == HCH - 1))
                        gs = bwrk.tile([P, TW], bf16, tag="gs", bufs=2, name="gs")
                        nc.scalar.activation(gs[:], gp[:], AF.Silu)
                        us = bwrk.tile([P, TW], bf16, tag="us", bufs=2, name="us")
                        nc.scalar.copy(us[:], up[:])
                        nc.vector.tensor_tensor(act[:, fi, :], us[:], gs[:], MUL)

                    if t == 0:
                        b_front1(3, bwrk, bwrk, bps, 1)
                        b_front2(2, bwrk, bwrk, bps, "gu", 4)
                    if t == 1:
                        b_front2(3, bwrk, bwrk, bps, "gu", 4)
                    if t >= 1:
                        b_fin(t - 1)

                    # down projection partial + RS2 (last chunk split in halves
                    # so the exposed tail collective is 1MB, not 2MB)
                    hos = list(range(HCH)) if t < TT - 1 else                         list(range(0, HCH, 2)) + list(range(1, HCH, 2))
                    for i, ho in enumerate(hos):
                        dpp = bps.tile([P, TW], f32, tag="d", bufs=2, name="dpp")
                        for c in range(FPC // P):
                            nc.tensor.matmul(dpp[:], wd_sb[:, c, ho * P:(ho + 1) * P],
                                             act[:, c, :],
                                             start=(c == 0), stop=(c == FPC // P - 1))
                        dsb = bwrk.tile([P, TW], bf16, tag="dsb", bufs=3, name="dsb")
                        if ho % 2 == 0:
                            nc.scalar.copy(dsb[:], dpp[:])
                        else:
                            nc.vector.tensor_copy(out=dsb[:], in_=dpp[:])
                        if t < TT - 1:
                            nc.sync.dma_start(rs2_in[t][ho * P:(ho + 1) * P, :],
                                              dsb[:])
                        else:
                            half = ho % 2
                            nc.sync.dma_start(
                                rs2h_in[half][(ho // 2) * P:(ho // 2 + 1) * P, :],
                                dsb[:])
                            if i == HCH // 2 - 1:
                                nc.gpsimd.collective_compute(
                                    "ReduceScatter", ADD,
                                    ins=[rs2h_in[0][:].opt()],
                                    outs=[rs2h_out[0][:].opt()], replica_groups=rg)
                    if t < TT - 1:
                        nc.gpsimd.collective_compute(
                            "ReduceScatter", ADD, ins=[rs2_in[t][:].opt()],
                            outs=[rs2_out[t][:].opt()], replica_groups=rg)
                    else:
                        nc.gpsimd.collective_compute(
                            "ReduceScatter", ADD, ins=[rs2h_in[1][:].opt()],
                            outs=[rs2h_out[1][:].opt()], replica_groups=rg)
                b_fin(TT - 1)
                bbig_box[0].release()
    nc.compile()
    return nc


def _prep(hidden_states, positions, w_in_ln, w_q, w_kv_a, w_kv_a_ln,
          w_kv_b, w_o, w_post_ln, w_gate, w_up, w_down):
    hT = np.ascontiguousarray(
        np.asarray(hidden_states, np.float32).reshape(T, HID).T)

    pos = np.asarray(positions).reshape(-1).astype(np.float64)
    inv = ROPE_BASE ** (-np.arange(0, DR, 2, dtype=np.float64) / DR)
    fr = pos[:, None] * inv[None, :]                      # [T, 32]
    c32 = np.cos(fr).T.astype(np.float32)                 # [32, T]
    s32 = np.sin(fr).T.astype(np.float32)
    cosf = np.concatenate([c32, c32], 0)                  # [64, T]
    sinf = np.concatenate([-s32, s32], 0)

    r = np.arange(P)[:, None]
    c = np.arange(TW)[None, :]
    masks = np.stack([np.where(c >= r + j * P, 0.0, NEG) for j in range(4)],
                     1).astype(np.float32)                # [128, 4, 512]

    w_in_ln = np.asarray(w_in_ln, np.float32)
    wqf = (np.asarray(w_q, np.float32) * w_in_ln[:, None] * SCALING
           ).reshape(HID, H, DQK)
    wkvaf = np.asarray(w_kv_a, np.float32) * w_in_ln[:, None]
    kpe_w = wkvaf[:, KVR:]
    wkva_p = np.concatenate([wkvaf[:, :KVR], kpe_w[:, 0::2], kpe_w[:, 1::2]], 1)
    wkvbf = (np.asarray(w_kv_b, np.float32)
             * np.asarray(w_kv_a_ln, np.float32)[:, None]).reshape(KVR, H, DN + DV)
    w_post_ln = np.asarray(w_post_ln, np.float32)
    wgf = np.asarray(w_gate, np.float32) * w_post_ln[:, None]
    wuf = np.asarray(w_up, np.float32) * w_post_ln[:, None]
    wdf = np.asarray(w_down, np.float32)
    wof = np.asarray(w_o, np.float32).reshape(H, DV, HID)

    hT_bf = hT.astype(BF)
    in_maps = []
    for core in range(NC_N):
        hs = [2 * core, 2 * core + 1]
        nopes = np.concatenate([wqf[:, h, :DN] for h in hs], 1)
        pes = []
        for h in hs:
            pe = wqf[:, h, DN:]
            pes += [pe[:, 0::2], pe[:, 1::2]]
        wq_c = np.concatenate([nopes] + pes, 1)
        in_maps.append({
            "htb": hT_bf,
            "hres": np.ascontiguousarray(hT_bf[core * HPR:(core + 1) * HPR, :]),
            "wq": wq_c.astype(BF),
            "wkva": wkva_p.astype(BF),
            "wkvbn": np.concatenate([wkvbf[:, h, :DN] for h in hs], 1).astype(BF),
            "wkvbv": np.concatenate([wkvbf[:, h, DN:] for h in hs], 1).astype(BF),
            "wo": np.concatenate([wof[h] for h in hs], 0).astype(BF),
            "wg": wgf[:, core * FPC:(core + 1) * FPC].astype(BF),
            "wu": wuf[:, core * FPC:(core + 1) * FPC].astype(BF),
            "wd": wdf[core * FPC:(core + 1) * FPC, :].astype(BF),
            "cosf": cosf.astype(BF),
            "sinf": sinf.astype(BF),
            "masks": masks.astype(BF),
        })
    return in_maps


def kernel(**inputs):
    if "nc" not in _CACHE:
        _CACHE["nc"] = _build()
    nc = _CACHE["nc"]
    in_maps = _prep(**inputs)
    res = run_bass_kernel_spmd(nc, in_maps, core_ids=list(range(NC_N)))
    outT = np.concatenate([np.asarray(res.results[c]["o"], np.float32)
                           for c in range(NC_N)], 0)
    return np.ascontiguousarray(outT.T).reshape(B, S, HID).astype(np.float32)
